# revision 20
# baseline (speedup 1.0000x reference)
"""Trainium2 Bass kernel for nn_BAAMamba (VMamba-style 4-direction Mamba classifier).

Sharding: pure data-parallel over batch - 8 cores x 1 image, each core runs the
full model on its image. No collectives.

v2 design (per-core, single NeuronCore), from measured op costs:
  - All cube elementwise ops in bf16 (DVE 2x mode); a-cube kept f32 so the
    scan decay chain has no compounding quantization.
  - n-reduction as a 4-level bf16 tree of tensor_adds (2.1us/ec) instead of
    strided tensor_reduce (5.6us/ec).
  - All matmuls in bf16 (weights host-cast), PSUM accumulates f32.
  - ACT ops clustered per depth by activation table; LN-stat ln/exp batched
    into one [128,8] op pair per depth (table reloads cost 1.3us each).
  - dt_b folded into the dt matmul via an appended ones row.
  - silu computed on ACT directly (Silu table) instead of sigmoid+DVE mul.
  - PSUM evacuations on ACT (copy/Identity-affine) to relieve DVE.
  - B/C broadcast cubes via bf16 DRAM round trip, double buffered.
  - Pool engine unused: it shares SBUF ports with DVE (measured 2-3x mutual
    slowdown) and cannot access PSUM; accumulating SWDGE DMA reduces race.
"""

import sys

import numpy as np

sys.path.insert(0, "/opt/trn_rl_repo")

import ml_dtypes  # noqa: E402

import concourse.bass as bass  # noqa: E402
import concourse.bacc as bacc  # noqa: E402
import concourse.tile as tile  # noqa: E402
from concourse import mybir  # noqa: E402

F32 = mybir.dt.float32
BF16 = mybir.dt.bfloat16
AF = mybir.ActivationFunctionType
ALU = mybir.AluOpType

B = 8
IMG = 224
PATCH = 16
D = 192
DEPTH = 8
H = IMG // PATCH
W = H
L = H * W                      # 196
D_IN = 384
N_ST = 16                      # D_STATE
DT_R = 12
NCLS = 1000
EPS = 1e-5

TS = [(0, 128), (128, L - 128)]          # t tiles (offset, size)
KD = [(0, 128), (128, D - 128)]          # d=192 contraction tiles
NE = D_IN // 128                         # 3 e-tiles
NDIR = 4


def build_nc(ndirs=NDIR, ndepth=DEPTH):
    nc = bacc.Bacc("TRN2")

    t_ = {}

    def din(name, shape, dt=BF16):
        t_[name] = nc.dram_tensor(name, shape, dt, kind="ExternalInput")

    din("xcol", (768, L))
    din("pwT", (768, D))
    din("pb", (D,), F32)
    din("pe_g", (D,), F32)
    din("pe_b", (D,), F32)
    din("lnwb", (4, DEPTH, D, 2), F32)           # [...,0]=w  [...,1]=b
    din("WinT", (4, DEPTH, D, 2 * D_IN))
    din("scal", (4, DEPTH, D_IN, 6), F32)        # convw(4) | convb | Dp
    din("WxT", (4, DEPTH, D_IN, DT_R + 2 * N_ST))
    din("dtw2", (4, DEPTH, DT_R + 1, D_IN))      # dt_w^T with dt_b row
    din("Aneg", (4, DEPTH, D_IN, N_ST), F32)
    din("WoT", (4, DEPTH, D_IN, D))
    din("onw", (D,), F32)
    din("onb", (D,), F32)
    din("hlw", (D,), F32)
    din("hlb", (D,), F32)
    din("hwT", (D, NCLS))
    din("hb", (NCLS,), F32)
    din("perm", (4, L, L))
    din("permI", (4, L, L))
    t_["logits"] = nc.dram_tensor("logits", (1, NCLS), F32, kind="ExternalOutput")

    with tile.TileContext(nc) as tc:
        _emit(nc, tc, t_, ndirs, ndepth)
    nc.compile()
    if not nc.is_finalized():
        nc.finalize()
    return nc


def _emit(nc, tc, t_, ndirs, ndepth):
    from contextlib import ExitStack

    with ExitStack() as ctx:
        consts = ctx.enter_context(tc.tile_pool(name="consts", bufs=1))
        wpool = ctx.enter_context(tc.tile_pool(name="wpool", bufs=2))
        state = ctx.enter_context(tc.tile_pool(name="state", bufs=1))
        apool = ctx.enter_context(tc.tile_pool(name="apool", bufs=2))
        small = ctx.enter_context(tc.tile_pool(name="small", bufs=3))
        cpool = ctx.enter_context(tc.tile_pool(name="cpool", bufs=2))
        ps1 = ctx.enter_context(tc.tile_pool(name="ps1", bufs=4, space="PSUM"))
        dpool = ctx.enter_context(tc.tile_pool(name="dpool", bufs=2, space="DRAM"))

        # ---- constants ----
        from concourse.masks import make_identity

        ident = consts.tile([128, 128], BF16)
        make_identity(nc, ident[:])
        identF = consts.tile([128, 128], F32)
        make_identity(nc, identF[:])

        pwT_sb = consts.tile([128, 6, D], BF16)
        col_sb = consts.tile([128, 6, L], BF16)
        for kt in range(6):
            nc.sync.dma_start(pwT_sb[:, kt, :], t_["pwT"][kt * 128:(kt + 1) * 128, :])
            nc.sync.dma_start(col_sb[:, kt, :], t_["xcol"][kt * 128:(kt + 1) * 128, :])

        P_sb = []
        PI_sb = []
        for di in range(ndirs):
            p = consts.tile([128, 2, L], BF16, tag=f"P{di}", name=f"P{di}")
            pi = consts.tile([128, 2, L], BF16, tag=f"PI{di}", name=f"PI{di}")
            for kt, (koff, ksz) in enumerate(TS):
                nc.sync.dma_start(p[:ksz, kt, :], t_["perm"][di, koff:koff + ksz, :])
                nc.sync.dma_start(pi[:ksz, kt, :], t_["permI"][di, koff:koff + ksz, :])
            P_sb.append(p)
            PI_sb.append(pi)

        def rep_vec(name):
            v = consts.tile([128, D], F32, tag=f"rep_{name}", name=f"rep_{name}")
            nc.sync.dma_start(v[:], t_[name][:].unsqueeze(0).broadcast_to((128, D)))
            return v

        pb_r = rep_vec("pb")
        peg_r = rep_vec("pe_g")
        peb_r = rep_vec("pe_b")
        onw_r = rep_vec("onw")
        onb_r = rep_vec("onb")
        hlw_r = rep_vec("hlw")
        hlb_r = rep_vec("hlb")

        hb_sb = consts.tile([1, NCLS], F32)
        nc.sync.dma_start(hb_sb[:], t_["hb"][:].unsqueeze(0))
        hwT_sb = consts.tile([128, 2, NCLS], BF16)
        for kd, (doff, dsz) in enumerate(KD):
            nc.sync.dma_start(hwT_sb[:dsz, kd, :], t_["hwT"][doff:doff + dsz, :])

        ones196 = consts.tile([1, L], BF16)
        nc.vector.memset(ones196[:], 1.0)
        onescol = consts.tile([128, 1], F32)
        nc.vector.memset(onescol[:], 1.0 / L)
        eps_t = consts.tile([128, 1], F32)
        nc.vector.memset(eps_t[:], EPS)

        # ---- helpers ----
        def emit_ln_stats(src_tt, tsz, tag):
            st6 = small.tile([128, 6], F32, tag="bn6", name="st6")
            mv = small.tile([128, 2], F32, tag=f"bn2_{tag}", name="mv")
            nc.vector.bn_stats(st6[:tsz], src_tt)
            nc.vector.bn_aggr(mv[:tsz], st6[:tsz])
            lnv = small.tile([128, 1], F32, tag="lnv", name="lnv")
            rstd = small.tile([128, 1], F32, tag=f"rstd_{tag}", name="rstd")
            nc.scalar.activation(lnv[:tsz], mv[:tsz, 1:2], AF.Ln, bias=eps_t[:tsz, :])
            nc.scalar.activation(rstd[:tsz], lnv[:tsz], AF.Exp, scale=-0.5)
            return mv, rstd

        def emit_ln(dst, src, tag):
            for tt, (toff, tsz) in enumerate(TS):
                mv, rstd = emit_ln_stats(src[:tsz, tt, :], tsz, tag)
                nc.vector.tensor_scalar(
                    out=dst[:tsz, tt, :], in0=src[:tsz, tt, :],
                    scalar1=mv[:tsz, 0:1], scalar2=rstd[:tsz, 0:1],
                    op0=ALU.subtract, op1=ALU.mult)

        # ---- patch embed ----
        feat_ln = state.tile([128, 2, D], F32, tag="feat_ln")
        for tt, (toff, tsz) in enumerate(TS):
            ps = ps1.tile([128, D], F32, tag="sps", name="ps")
            for kt in range(6):
                nc.tensor.matmul(ps[:tsz, :], col_sb[:, kt, toff:toff + tsz],
                                 pwT_sb[:, kt, :], start=(kt == 0), stop=(kt == 5))
            nc.vector.tensor_add(feat_ln[:tsz, tt, :], ps[:tsz, :], pb_r[:tsz, :])
        xhat0 = state.tile([128, 2, D], F32, tag="xhat0")
        emit_ln(xhat0, feat_ln, "pe")
        for tt, (toff, tsz) in enumerate(TS):
            nc.vector.tensor_mul(feat_ln[:tsz, tt, :], xhat0[:tsz, tt, :], peg_r[:tsz, :])
            nc.vector.tensor_add(feat_ln[:tsz, tt, :], feat_ln[:tsz, tt, :], peb_r[:tsz, :])
        # shared depth-0 block-LN xhat of feat_ln, in bf16 for the perm matmuls
        emit_ln(xhat0, feat_ln, "blk0")
        xhat0_16 = state.tile([128, 2, D], BF16, tag="xhat0_16")
        for tt, (toff, tsz) in enumerate(TS):
            nc.vector.tensor_scalar_mul(xhat0_16[:tsz, tt, :], xhat0[:tsz, tt, :], 1.0)

        # ---- per-direction persistent state ----
        res_t = [state.tile([128, 2, D], F32, tag=f"res{di}", name=f"res{di}")
                 for di in range(ndirs)]
        # residual init: res[di] = perm_di(feat_ln)
        feat16 = state.tile([128, 2, D], BF16, tag="feat16")
        for tt, (toff, tsz) in enumerate(TS):
            nc.vector.tensor_scalar_mul(feat16[:tsz, tt, :], feat_ln[:tsz, tt, :], 1.0)
        for di in range(ndirs):
            for tt, (toff, tsz) in enumerate(TS):
                ps = ps1.tile([128, D], F32, tag="sps", name="ps")
                for kt, (koff, ksz) in enumerate(TS):
                    nc.tensor.matmul(ps[:tsz, :], P_sb[di][:ksz, kt, toff:toff + tsz],
                                     feat16[:ksz, kt, :], start=(kt == 0), stop=(kt == 1))
                nc.scalar.copy(res_t[di][:tsz, tt, :], ps[:tsz, :])

        # ================= depth loop, phase-clustered across directions ====
        mstate = [dict() for _ in range(ndirs)]

        for dep in range(ndepth):
            # --- A1: ln weights + residual update + LN xhat -> xlnT ---------
            for di in range(ndirs):
                st = mstate[di]
                st["lnwb"] = wpool.tile([128, 2, 2], F32, tag="lnwb", bufs=5,
                                        name="lnwb")
                for kd, (doff, dsz) in enumerate(KD):
                    nc.sync.dma_start(st["lnwb"][:dsz, kd, :],
                                      t_["lnwb"][di, dep, doff:doff + dsz, :])
                st["xlnT"] = apool.tile([128, 2, L], BF16, tag="xlnT", bufs=5,
                                        name="xlnT")

            if dep > 0:
                stats = []
                for di in range(ndirs):
                    res = res_t[di]
                    for tt, (toff, tsz) in enumerate(TS):
                        st6 = small.tile([128, 6], F32, tag="bn6", name="st6")
                        mv = small.tile([128, 2], F32, tag=f"bn2_{di}_{tt}", name="mv")
                        nc.vector.bn_stats(st6[:tsz], res[:tsz, tt, :])
                        nc.vector.bn_aggr(mv[:tsz], st6[:tsz])
                        stats.append(mv)
                # gather 8 vars -> one Ln + one Exp (one table load each)
                vars8 = small.tile([128, 8], F32, tag="vars8", name="vars8")
                for i, mv in enumerate(stats):
                    tsz = TS[i % 2][1]
                    nc.vector.tensor_scalar_add(vars8[:tsz, i:i + 1], mv[:tsz, 1:2],
                                                EPS)
                lnv8 = small.tile([128, 8], F32, tag="lnv8", name="lnv8")
                nc.scalar.activation(lnv8[:], vars8[:], AF.Ln)
                rstd8 = small.tile([128, 8], F32, tag="rstd8", name="rstd8")
                nc.scalar.activation(rstd8[:], lnv8[:], AF.Exp, scale=-0.5)
                # negmr[i] = -mean_i * rstd_i  (tiny DVE ops)
                negmr = small.tile([128, 8], F32, tag="negmr", name="negmr")
                for i, mv in enumerate(stats):
                    tsz = TS[i % 2][1]
                    nc.vector.tensor_scalar(
                        out=negmr[:tsz, i:i + 1], in0=mv[:tsz, 0:1],
                        scalar1=rstd8[:tsz, i:i + 1], scalar2=-1.0,
                        op0=ALU.mult, op1=ALU.mult)

            # ---- grouped per-direction pipeline: dir 0 first, then 1-3 -----
            for grp in ([0], list(range(1, ndirs))):
                # LN xhat -> transpose -> xlnT
                for di in grp:
                    st = mstate[di]
                    if dep == 0:
                        for kd, (doff, dsz) in enumerate(KD):
                            ps = ps1.tile([128, L], F32, tag="sps", name="ps")
                            for kt, (koff, ksz) in enumerate(TS):
                                nc.tensor.matmul(ps[:dsz, :],
                                                 xhat0_16[:ksz, kt, doff:doff + dsz],
                                                 P_sb[di][:ksz, kt, :],
                                                 start=(kt == 0), stop=(kt == 1))
                            nc.scalar.activation(
                                st["xlnT"][:dsz, kd, :], ps[:dsz, :], AF.Identity,
                                scale=st["lnwb"][:dsz, kd, 0:1],
                                bias=st["lnwb"][:dsz, kd, 1:2])
                    else:
                        res = res_t[di]
                        xh16 = apool.tile([128, 2, D], BF16, tag="xh16", bufs=4)
                        for tt, (toff, tsz) in enumerate(TS):
                            i = di * 2 + tt
                            nc.scalar.activation(
                                xh16[:tsz, tt, :], res[:tsz, tt, :], AF.Identity,
                                scale=rstd8[:tsz, i:i + 1], bias=negmr[:tsz, i:i + 1])
                        for kd, (doff, dsz) in enumerate(KD):
                            ps = ps1.tile([128, L], BF16, tag="spsT", bufs=2,
                                          name="psT")
                            for tt, (toff, tsz) in enumerate(TS):
                                nc.tensor.transpose(ps[:dsz, toff:toff + tsz],
                                                    xh16[:tsz, tt, doff:doff + dsz],
                                                    ident[:tsz, :tsz])
                            nc.scalar.activation(
                                st["xlnT"][:dsz, kd, :], ps[:dsz, :], AF.Identity,
                                scale=st["lnwb"][:dsz, kd, 0:1],
                                bias=st["lnwb"][:dsz, kd, 1:2])

                # in_proj + conv + z evac
                for di in grp:
                    st = mstate[di]
                    WinT_sb = wpool.tile([128, 2, 2 * D_IN], BF16, tag="WinT", bufs=2)
                    for kd, (doff, dsz) in enumerate(KD):
                        nc.sync.dma_start(WinT_sb[:dsz, kd, :],
                                          t_["WinT"][di, dep, doff:doff + dsz, :])
                    scal_sb = wpool.tile([128, NE, 6], F32, tag="scal", bufs=5)
                    nc.sync.dma_start(
                        scal_sb[:],
                        t_["scal"][di, dep].rearrange("(a p) s -> p a s", p=128))
                    st["scal"] = scal_sb

                    accz = apool.tile([128, 2, NE, L], BF16, tag="accz", bufs=4)
                    acc = accz[:, 0]
                    zsb = accz[:, 1]
                    for ec in range(2 * NE):
                        ps = ps1.tile([128, L], F32, tag="sps", name="ps")
                        for kd, (doff, dsz) in enumerate(KD):
                            nc.tensor.matmul(ps[:, :],
                                             WinT_sb[:dsz, kd, ec * 128:(ec + 1) * 128],
                                             st["xlnT"][:dsz, kd, :],
                                             start=(kd == 0), stop=(kd == 1))
                        if ec < NE:
                            # causal depthwise conv reading u straight from PSUM
                            nc.vector.tensor_scalar(
                                out=acc[:, ec, :], in0=ps[:, :],
                                scalar1=scal_sb[:, ec, 3:4],
                                scalar2=scal_sb[:, ec, 4:5],
                                op0=ALU.mult, op1=ALU.add)
                            for k in range(1, 4):
                                nc.vector.scalar_tensor_tensor(
                                    out=acc[:, ec, k:L], in0=ps[:, 0:L - k],
                                    scalar=scal_sb[:, ec, 3 - k:4 - k],
                                    in1=acc[:, ec, k:L], op0=ALU.mult, op1=ALU.add)
                        else:
                            nc.vector.tensor_scalar_mul(zsb[:, ec - NE, :],
                                                        ps[:, :], 1.0)
                    st["accz"] = accz

                # silu gates
                for di in grp:
                    st = mstate[di]
                    u2sz = apool.tile([128, 2, NE, L], BF16, tag="u2sz", bufs=5)
                    nc.scalar.activation(u2sz[:].rearrange("p s a t -> p (s a t)"),
                                         st["accz"][:].rearrange("p s a t -> p (s a t)"),
                                         AF.Silu)
                    st["u2"] = u2sz[:, 0]
                    st["sz"] = u2sz[:, 1]

                # x_proj B/C + round trips, then dt
                for di in grp:
                    st = mstate[di]
                    WxT_sb = wpool.tile([128, NE, DT_R + 2 * N_ST], BF16, tag="WxT",
                                        bufs=5)
                    dtw2_sb = wpool.tile([DT_R + 1, NE, 128], BF16, tag="dtw2",
                                         bufs=5)
                    for ke in range(NE):
                        nc.sync.dma_start(
                            WxT_sb[:, ke, :],
                            t_["WxT"][di, dep, ke * 128:(ke + 1) * 128, :])
                        nc.sync.dma_start(
                            dtw2_sb[:, ke, :],
                            t_["dtw2"][di, dep, :, ke * 128:(ke + 1) * 128])
                    st["dtw2"] = dtw2_sb
                    st["WxT"] = WxT_sb

                    Bsb = small.tile([N_ST, L], BF16, tag="Bsb", bufs=4)
                    Csb = small.tile([N_ST, L], BF16, tag="Csb", bufs=4)
                    for si, (soff, ssz) in [(1, (DT_R, N_ST)),
                                            (2, (DT_R + N_ST, N_ST))]:
                        psx = ps1.tile([N_ST, L], F32, tag="spsx", bufs=2, name="psx")
                        for ke in range(NE):
                            nc.tensor.matmul(psx[:ssz, :],
                                             WxT_sb[:, ke, soff:soff + ssz],
                                             st["u2"][:, ke, :], start=(ke == 0),
                                             stop=(ke == NE - 1))
                        dst = Bsb if si == 1 else Csb
                        nc.scalar.copy(dst[:ssz, :], psx[:ssz, :])

                    bc_dram = dpool.tile([2, N_ST * L], BF16, tag="bc_dram", bufs=4)
                    nc.sync.dma_start(
                        bc_dram[0:1, :].rearrange("a (n t) -> (a n) t", t=L), Bsb[:, :])
                    nc.sync.dma_start(
                        bc_dram[1:2, :].rearrange("a (n t) -> (a n) t", t=L), Csb[:, :])
                    B_r = cpool.tile([128, N_ST, L], BF16, tag="Brep", bufs=3)
                    C_r = cpool.tile([128, N_ST, L], BF16, tag="Crep", bufs=2)
                    nc.sync.dma_start(B_r[:].rearrange("p n t -> p (n t)"),
                                      bc_dram[0:1, :].broadcast_to((128, N_ST * L)))
                    nc.sync.dma_start(C_r[:].rearrange("p n t -> p (n t)"),
                                      bc_dram[1:2, :].broadcast_to((128, N_ST * L)))
                    st["B_r"] = B_r
                    st["C_r"] = C_r

                    dtm = apool.tile([DT_R + 1, L], BF16, tag="dtm", bufs=5)
                    nc.sync.dma_start(dtm[DT_R:DT_R + 1, :], ones196[:])
                    psx = ps1.tile([N_ST, L], F32, tag="spsx", bufs=2, name="psx")
                    for ke in range(NE):
                        nc.tensor.matmul(psx[:DT_R, :], st["WxT"][:, ke, 0:DT_R],
                                         st["u2"][:, ke, :], start=(ke == 0),
                                         stop=(ke == NE - 1))
                    nc.scalar.copy(dtm[:DT_R, :], psx[:DT_R, :])
                    st["dtm"] = dtm

                # softplus delta + v
                for di in grp:
                    st = mstate[di]
                    Aneg_sb = wpool.tile([128, NE, N_ST], F32, tag="Aneg", bufs=5)
                    nc.sync.dma_start(
                        Aneg_sb[:],
                        t_["Aneg"][di, dep].rearrange("(a p) n -> p a n", p=128))
                    st["Aneg"] = Aneg_sb
                    spe = apool.tile([128, NE, L], BF16, tag="spe", bufs=4)
                    for ec in range(NE):
                        psd = ps1.tile([128, L], F32, tag="sps", name="ps")
                        nc.tensor.matmul(psd[:, :], st["dtw2"][:, ec, :],
                                         st["dtm"][:, :], start=True, stop=True)
                        nc.scalar.activation(spe[:, ec, :], psd[:, :], AF.Exp)
                    st["spe"] = spe
                for di in grp:
                    st = mstate[di]
                    delta = apool.tile([128, NE, L], BF16, tag="delta", bufs=5)
                    nc.scalar.activation(delta[:].rearrange("p a t -> p (a t)"),
                                         st["spe"][:].rearrange("p a t -> p (a t)"),
                                         AF.Ln, bias=1.0)
                    st["delta"] = delta
                for di in grp:
                    st = mstate[di]
                    v = apool.tile([128, NE, L], BF16, tag="v", bufs=5)
                    nc.vector.tensor_mul(v[:].rearrange("p a t -> p (a t)"),
                                         st["delta"][:].rearrange("p a t -> p (a t)"),
                                         st["u2"].rearrange("p a t -> p (a t)"))
                    st["v"] = v

            # --- B3: a-cube + scan + y (per ec, DVE+ACT pipelined) ----------
            for di in range(ndirs):
                st = mstate[di]
                y_sb = apool.tile([128, NE, L], BF16, tag="ysb", bufs=5)
                for ec in range(NE):
                    a_sb = cpool.tile([128, N_ST, L], F32, tag="acube", bufs=2)
                    nc.vector.memset(a_sb[:, :, 0:1], 0.0)
                    if di == 0 and ec == 0:
                        # boundary-fill path: da on DVE, one big exp on ACT
                        for n in range(N_ST):
                            nc.vector.tensor_scalar_mul(
                                a_sb[:, n, 1:L], st["delta"][:, ec, 1:L],
                                st["Aneg"][:, ec, n:n + 1])
                        nc.scalar.activation(a_sb[:, :, 1:L], a_sb[:, :, 1:L],
                                             AF.Exp)
                    else:
                        for n in range(N_ST):
                            nc.scalar.activation(a_sb[:, n, 1:L],
                                                 st["delta"][:, ec, 1:L],
                                                 AF.Exp, scale=st["Aneg"][:, ec, n:n + 1])
                    b_sb = cpool.tile([128, N_ST, L], BF16, tag="bg", bufs=2)
                    v_b = st["v"][:, ec, :].unsqueeze(1).broadcast_to((128, N_ST, L))
                    nc.vector.tensor_mul(b_sb[:], v_b, st["B_r"][:])
                    h_sb = cpool.tile([128, N_ST, L], BF16, tag="hcube", bufs=1)
                    nc.vector.tensor_tensor_scan(
                        out=h_sb[:].rearrange("p n t -> p (n t)"),
                        data0=a_sb[:].rearrange("p n t -> p (n t)"),
                        data1=b_sb[:].rearrange("p n t -> p (n t)"),
                        initial=0.0, op0=ALU.mult, op1=ALU.add)
                    g_sb = cpool.tile([128, N_ST, L], BF16, tag="bg", bufs=2)
                    nc.vector.tensor_mul(g_sb[:], h_sb[:], st["C_r"][:])
                    nc.vector.tensor_add(g_sb[:, 0:8, :], g_sb[:, 0:8, :],
                                         g_sb[:, 8:16, :])
                    nc.vector.tensor_add(g_sb[:, 0:4, :], g_sb[:, 0:4, :],
                                         g_sb[:, 4:8, :])
                    nc.vector.tensor_add(g_sb[:, 0:2, :], g_sb[:, 0:2, :],
                                         g_sb[:, 2:4, :])
                    nc.vector.tensor_add(y_sb[:, ec, :], g_sb[:, 0, :], g_sb[:, 1, :])
                st["y_sb"] = y_sb

            # --- B4: gate + out_proj -> hidden ------------------------------
            for di in range(ndirs):
                st = mstate[di]
                WoT_sb = wpool.tile([128, NE, D], BF16, tag="WoT", bufs=2)
                for ke in range(NE):
                    nc.sync.dma_start(WoT_sb[:, ke, :],
                                      t_["WoT"][di, dep, ke * 128:(ke + 1) * 128, :])
                y_sb = st["y_sb"]
                for ec in range(NE):
                    nc.vector.scalar_tensor_tensor(
                        out=y_sb[:, ec, :], in0=st["u2"][:, ec, :],
                        scalar=st["scal"][:, ec, 5:6], in1=y_sb[:, ec, :],
                        op0=ALU.mult, op1=ALU.add)
                nc.vector.tensor_mul(y_sb[:].rearrange("p a t -> p (a t)"),
                                     y_sb[:].rearrange("p a t -> p (a t)"),
                                     st["sz"].rearrange("p a t -> p (a t)"))
                for tt, (toff, tsz) in enumerate(TS):
                    pso = ps1.tile([128, D], F32, tag="sps", name="ps")
                    for ke in range(NE):
                        nc.tensor.matmul(pso[:tsz, :], y_sb[:, ke, toff:toff + tsz],
                                         WoT_sb[:, ke, :], start=(ke == 0),
                                         stop=(ke == NE - 1))
                    # residual update directly from PSUM (replaces hid tiles)
                    nc.vector.tensor_add(res_t[di][:tsz, tt, :],
                                         res_t[di][:tsz, tt, :], pso[:tsz, :])

        # ---- final = hidden + residual ; CrossMerge ----
        res16 = []
        for di in range(ndirs):
            r16 = apool.tile([128, 2, D], BF16, tag="res16", bufs=4, name="r16")
            for tt, (toff, tsz) in enumerate(TS):
                nc.vector.tensor_scalar_mul(r16[:tsz, tt, :], res_t[di][:tsz, tt, :],
                                            1.0)
            res16.append(r16)
        merged = state.tile([128, 2, D], F32, tag="merged")
        for tt, (toff, tsz) in enumerate(TS):
            ps = ps1.tile([128, D], F32, tag="sps", name="ps")
            nmm = ndirs * 2
            i = 0
            for di in range(ndirs):
                for kt, (koff, ksz) in enumerate(TS):
                    nc.tensor.matmul(ps[:tsz, :], PI_sb[di][:ksz, kt, toff:toff + tsz],
                                     res16[di][:ksz, kt, :], start=(i == 0),
                                     stop=(i == nmm - 1))
                    i += 1
            nc.scalar.copy(merged[:tsz, tt, :], ps[:tsz, :])

        # ---- out_norm LN + head LN ----
        xh = state.tile([128, 2, D], F32, tag="xh_final")
        emit_ln(xh, merged, "on")
        for tt, (toff, tsz) in enumerate(TS):
            nc.vector.tensor_mul(merged[:tsz, tt, :], xh[:tsz, tt, :], onw_r[:tsz, :])
            nc.vector.tensor_add(merged[:tsz, tt, :], merged[:tsz, tt, :], onb_r[:tsz, :])
        emit_ln(xh, merged, "hl")
        for tt, (toff, tsz) in enumerate(TS):
            nc.vector.tensor_mul(merged[:tsz, tt, :], xh[:tsz, tt, :], hlw_r[:tsz, :])
            nc.vector.tensor_add(merged[:tsz, tt, :], merged[:tsz, tt, :], hlb_r[:tsz, :])

        # ---- mean pool (x 1/L via ones value) ----
        psp = ps1.tile([1, D], F32, tag="spsx", bufs=2, name="psx")
        for kt, (koff, ksz) in enumerate(TS):
            nc.tensor.matmul(psp[:, :], onescol[:ksz, :], merged[:ksz, kt, :],
                             start=(kt == 0), stop=(kt == 1))
        pooled = small.tile([1, D], F32, tag="pooled")
        nc.scalar.copy(pooled[:], psp[:])
        pooledT = small.tile([128, 2, 1], BF16, tag="pooledT")
        for kd, (doff, dsz) in enumerate(KD):
            pst = ps1.tile([128, 1], F32, tag="spsx", bufs=2, name="psx")
            nc.tensor.transpose(pst[:dsz, :], pooled[:, doff:doff + dsz], identF[:1, :1])
            nc.scalar.copy(pooledT[:dsz, kd, :], pst[:dsz, :])

        # ---- head ----
        log_sb = small.tile([1, NCLS], F32, tag="logsb")
        for half in range(2):
            psh = ps1.tile([1, 500], F32, tag="spsx", bufs=2, name="psx")
            for kd, (doff, dsz) in enumerate(KD):
                nc.tensor.matmul(psh[:, :], pooledT[:dsz, kd, :],
                                 hwT_sb[:dsz, kd, half * 500:(half + 1) * 500],
                                 start=(kd == 0), stop=(kd == 1))
            nc.vector.tensor_add(log_sb[:, half * 500:(half + 1) * 500], psh[:, :],
                                 hb_sb[:, half * 500:(half + 1) * 500])
        nc.sync.dma_start(t_["logits"][:], log_sb[:])


# ============================== host side ==============================

_NC_CACHE = {}


def _get_nc():
    if "nc" not in _NC_CACHE:
        _NC_CACHE["nc"] = build_nc()
    return _NC_CACHE["nc"]


def _perm_matrices():
    idx = np.arange(L).reshape(H, W)
    perm0 = idx.reshape(-1)
    perm1 = idx.T.reshape(-1)
    perms = [perm0, perm1, perm0[::-1].copy(), perm1[::-1].copy()]
    P = np.zeros((4, L, L), np.float32)
    PI = np.zeros((4, L, L), np.float32)
    for di, pm in enumerate(perms):
        P[di, pm, np.arange(L)] = 1.0       # seq[t'] = sum_t P[t,t'] feat[t]
        PI[di] = P[di].T                     # merged[t] = sum_t' PI[t',t] out[t']
    return P, PI


def prep_inputs(inputs):
    """Host-side layout prep. Returns (shared weight map, per-core xcol list)."""
    bf = ml_dtypes.bfloat16
    g = {k: np.ascontiguousarray(np.asarray(v, dtype=np.float32))
         for k, v in inputs.items()}
    P, PI = _perm_matrices()
    lnwb = np.stack([g["ln_w"], g["ln_b"]], axis=-1)          # (4,8,192,2)
    scal = np.concatenate([g["conv_w"], g["conv_b"][..., None],
                           g["Dp"][..., None]], axis=-1)      # (4,8,384,6)
    dtw2 = np.concatenate([g["dt_w"].transpose(0, 1, 3, 2),
                           g["dt_b"][:, :, None, :]], axis=2)  # (4,8,13,384)
    shared = dict(
        pwT=np.ascontiguousarray(g["patch_w"].reshape(D, 768).T).astype(bf),
        pb=g["patch_b"], pe_g=g["pe_ln_w"], pe_b=g["pe_ln_b"],
        lnwb=np.ascontiguousarray(lnwb),
        WinT=np.ascontiguousarray(g["in_proj_w"].transpose(0, 1, 3, 2)).astype(bf),
        scal=np.ascontiguousarray(scal),
        WxT=np.ascontiguousarray(g["x_proj_w"].transpose(0, 1, 3, 2)).astype(bf),
        dtw2=np.ascontiguousarray(dtw2).astype(bf),
        Aneg=np.ascontiguousarray(-np.exp(g["A_log"])),
        WoT=np.ascontiguousarray(g["out_proj_w"].transpose(0, 1, 3, 2)).astype(bf),
        onw=g["out_norm_w"], onb=g["out_norm_b"],
        hlw=g["head_ln_w"], hlb=g["head_ln_b"],
        hwT=np.ascontiguousarray(g["head_w"].T).astype(bf), hb=g["head_b"],
        perm=P.astype(bf), permI=PI.astype(bf),
    )
    x = g["x"]
    xcols = []
    for b in range(x.shape[0]):
        xb = x[b]                                          # (3, 224, 224)
        c = xb.reshape(3, H, PATCH, W, PATCH)              # (3, i, pi, j, pj)
        col = c.transpose(0, 2, 4, 1, 3).reshape(768, L)   # (c,pi,pj),(i,j)
        xcols.append(np.ascontiguousarray(col).astype(bf))
    return shared, xcols


def kernel(**inputs):
    from concourse.bass_utils import run_bass_kernel_spmd

    nc = _get_nc()
    shared, xcols = prep_inputs(inputs)
    nb = len(xcols)
    in_maps = [dict(shared, xcol=xcols[b]) for b in range(nb)]
    res = run_bass_kernel_spmd(nc, in_maps, core_ids=list(range(nb)))
    out = np.stack([res.results[b]["logits"][0] for b in range(nb)])
    return out.astype(np.float32)


# revision 23
# speedup vs baseline: 1.0212x; 1.0212x over previous
"""Trainium2 Bass kernel for nn_BAAMamba (VMamba-style 4-direction Mamba classifier).

Sharding: pure data-parallel over batch - 8 cores x 1 image, each core runs the
full model on its image. No collectives.

v2 design (per-core, single NeuronCore), from measured op costs:
  - All cube elementwise ops in bf16 (DVE 2x mode); a-cube kept f32 so the
    scan decay chain has no compounding quantization.
  - n-reduction as a 4-level bf16 tree of tensor_adds (2.1us/ec) instead of
    strided tensor_reduce (5.6us/ec).
  - All matmuls in bf16 (weights host-cast), PSUM accumulates f32.
  - ACT ops clustered per depth by activation table; LN-stat ln/exp batched
    into one [128,8] op pair per depth (table reloads cost 1.3us each).
  - dt_b folded into the dt matmul via an appended ones row.
  - silu computed on ACT directly (Silu table) instead of sigmoid+DVE mul.
  - PSUM evacuations on ACT (copy/Identity-affine) to relieve DVE.
  - B/C broadcast cubes via bf16 DRAM round trip, double buffered.
  - Pool engine unused: it shares SBUF ports with DVE (measured 2-3x mutual
    slowdown) and cannot access PSUM; accumulating SWDGE DMA reduces race.
"""

import sys

import numpy as np

sys.path.insert(0, "/opt/trn_rl_repo")

import ml_dtypes  # noqa: E402

import concourse.bass as bass  # noqa: E402
import concourse.bacc as bacc  # noqa: E402
import concourse.tile as tile  # noqa: E402
from concourse import mybir  # noqa: E402

F32 = mybir.dt.float32
BF16 = mybir.dt.bfloat16
AF = mybir.ActivationFunctionType
ALU = mybir.AluOpType

B = 8
IMG = 224
PATCH = 16
D = 192
DEPTH = 8
H = IMG // PATCH
W = H
L = H * W                      # 196
D_IN = 384
N_ST = 16                      # D_STATE
DT_R = 12
NCLS = 1000
EPS = 1e-5

TS = [(0, 128), (128, L - 128)]          # t tiles (offset, size)
KD = [(0, 128), (128, D - 128)]          # d=192 contraction tiles
NE = D_IN // 128                         # 3 e-tiles
NDIR = 4


def build_nc(ndirs=NDIR, ndepth=DEPTH):
    nc = bacc.Bacc("TRN2")

    t_ = {}

    def din(name, shape, dt=BF16):
        t_[name] = nc.dram_tensor(name, shape, dt, kind="ExternalInput")

    din("xcol", (768, L))
    din("pwT", (768, D))
    din("pb", (D,), F32)
    din("pe_g", (D,), F32)
    din("pe_b", (D,), F32)
    din("lnwb", (4, DEPTH, D, 2), F32)           # [...,0]=w  [...,1]=b
    din("WinT", (4, DEPTH, D, 2 * D_IN))
    din("scal", (4, DEPTH, D_IN, 6), F32)        # convw(4) | convb | Dp
    din("WxT", (4, DEPTH, D_IN, DT_R + 2 * N_ST))
    din("dtw2", (4, DEPTH, DT_R + 1, D_IN))      # dt_w^T with dt_b row
    din("Aneg", (4, DEPTH, D_IN, N_ST), F32)
    din("WoT", (4, DEPTH, D_IN, D))
    din("onw", (D,), F32)
    din("onb", (D,), F32)
    din("hlw", (D,), F32)
    din("hlb", (D,), F32)
    din("hwT", (D, NCLS))
    din("hb", (NCLS,), F32)
    din("perm", (4, L, L))
    din("permI", (4, L, L))
    t_["logits"] = nc.dram_tensor("logits", (1, NCLS), F32, kind="ExternalOutput")

    with tile.TileContext(nc) as tc:
        _emit(nc, tc, t_, ndirs, ndepth)
    nc.compile()
    if not nc.is_finalized():
        nc.finalize()
    return nc


def _emit(nc, tc, t_, ndirs, ndepth):
    from contextlib import ExitStack

    with ExitStack() as ctx:
        consts = ctx.enter_context(tc.tile_pool(name="consts", bufs=1))
        wpool = ctx.enter_context(tc.tile_pool(name="wpool", bufs=2))
        state = ctx.enter_context(tc.tile_pool(name="state", bufs=1))
        apool = ctx.enter_context(tc.tile_pool(name="apool", bufs=2))
        small = ctx.enter_context(tc.tile_pool(name="small", bufs=3))
        cpool = ctx.enter_context(tc.tile_pool(name="cpool", bufs=2))
        ps1 = ctx.enter_context(tc.tile_pool(name="ps1", bufs=4, space="PSUM"))
        dpool = ctx.enter_context(tc.tile_pool(name="dpool", bufs=2, space="DRAM"))

        # ---- constants ----
        from concourse.masks import make_identity

        ident = consts.tile([128, 128], BF16)
        make_identity(nc, ident[:])
        identF = consts.tile([128, 128], F32)
        make_identity(nc, identF[:])

        pwT_sb = consts.tile([128, 6, D], BF16)
        col_sb = consts.tile([128, 6, L], BF16)
        for kt in range(6):
            nc.sync.dma_start(pwT_sb[:, kt, :], t_["pwT"][kt * 128:(kt + 1) * 128, :])
            nc.sync.dma_start(col_sb[:, kt, :], t_["xcol"][kt * 128:(kt + 1) * 128, :])

        P_sb = []
        PI_sb = []
        for di in range(ndirs):
            p = consts.tile([128, 2, L], BF16, tag=f"P{di}", name=f"P{di}")
            pi = consts.tile([128, 2, L], BF16, tag=f"PI{di}", name=f"PI{di}")
            for kt, (koff, ksz) in enumerate(TS):
                nc.sync.dma_start(p[:ksz, kt, :], t_["perm"][di, koff:koff + ksz, :])
                nc.sync.dma_start(pi[:ksz, kt, :], t_["permI"][di, koff:koff + ksz, :])
            P_sb.append(p)
            PI_sb.append(pi)

        def rep_vec(name):
            v = consts.tile([128, D], F32, tag=f"rep_{name}", name=f"rep_{name}")
            nc.sync.dma_start(v[:], t_[name][:].unsqueeze(0).broadcast_to((128, D)))
            return v

        pb_r = rep_vec("pb")
        peg_r = rep_vec("pe_g")
        peb_r = rep_vec("pe_b")
        onw_r = rep_vec("onw")
        onb_r = rep_vec("onb")
        hlw_r = rep_vec("hlw")
        hlb_r = rep_vec("hlb")

        hb_sb = consts.tile([1, NCLS], F32)
        nc.sync.dma_start(hb_sb[:], t_["hb"][:].unsqueeze(0))
        hwT_sb = consts.tile([128, 2, NCLS], BF16)
        for kd, (doff, dsz) in enumerate(KD):
            nc.sync.dma_start(hwT_sb[:dsz, kd, :], t_["hwT"][doff:doff + dsz, :])

        ones196 = consts.tile([1, L], BF16)
        nc.vector.memset(ones196[:], 1.0)
        onescol = consts.tile([128, 1], F32)
        nc.vector.memset(onescol[:], 1.0 / L)
        eps_t = consts.tile([128, 1], F32)
        nc.vector.memset(eps_t[:], EPS)

        # ---- helpers ----
        def emit_ln_stats(src_tt, tsz, tag):
            st6 = small.tile([128, 6], F32, tag="bn6", name="st6")
            mv = small.tile([128, 2], F32, tag=f"bn2_{tag}", name="mv")
            nc.vector.bn_stats(st6[:tsz], src_tt)
            nc.vector.bn_aggr(mv[:tsz], st6[:tsz])
            lnv = small.tile([128, 1], F32, tag="lnv", name="lnv")
            rstd = small.tile([128, 1], F32, tag=f"rstd_{tag}", name="rstd")
            nc.scalar.activation(lnv[:tsz], mv[:tsz, 1:2], AF.Ln, bias=eps_t[:tsz, :])
            nc.scalar.activation(rstd[:tsz], lnv[:tsz], AF.Exp, scale=-0.5)
            return mv, rstd

        def emit_ln(dst, src, tag):
            for tt, (toff, tsz) in enumerate(TS):
                mv, rstd = emit_ln_stats(src[:tsz, tt, :], tsz, tag)
                nc.vector.tensor_scalar(
                    out=dst[:tsz, tt, :], in0=src[:tsz, tt, :],
                    scalar1=mv[:tsz, 0:1], scalar2=rstd[:tsz, 0:1],
                    op0=ALU.subtract, op1=ALU.mult)

        # ---- patch embed ----
        feat_ln = state.tile([128, 2, D], F32, tag="feat_ln")
        for tt, (toff, tsz) in enumerate(TS):
            ps = ps1.tile([128, D], F32, tag="sps", name="ps")
            for kt in range(6):
                nc.tensor.matmul(ps[:tsz, :], col_sb[:, kt, toff:toff + tsz],
                                 pwT_sb[:, kt, :], start=(kt == 0), stop=(kt == 5))
            nc.vector.tensor_add(feat_ln[:tsz, tt, :], ps[:tsz, :], pb_r[:tsz, :])
        xhat0 = state.tile([128, 2, D], F32, tag="xhat0")
        emit_ln(xhat0, feat_ln, "pe")
        for tt, (toff, tsz) in enumerate(TS):
            nc.vector.tensor_mul(feat_ln[:tsz, tt, :], xhat0[:tsz, tt, :], peg_r[:tsz, :])
            nc.vector.tensor_add(feat_ln[:tsz, tt, :], feat_ln[:tsz, tt, :], peb_r[:tsz, :])
        # shared depth-0 block-LN xhat of feat_ln, in bf16 for the perm matmuls
        emit_ln(xhat0, feat_ln, "blk0")
        xhat0_16 = state.tile([128, 2, D], BF16, tag="xhat0_16")
        for tt, (toff, tsz) in enumerate(TS):
            nc.vector.tensor_scalar_mul(xhat0_16[:tsz, tt, :], xhat0[:tsz, tt, :], 1.0)

        # ---- per-direction persistent state ----
        res_t = [state.tile([128, 2, D], F32, tag=f"res{di}", name=f"res{di}")
                 for di in range(ndirs)]
        # residual init: res[di] = perm_di(feat_ln)
        feat16 = state.tile([128, 2, D], BF16, tag="feat16")
        for tt, (toff, tsz) in enumerate(TS):
            nc.vector.tensor_scalar_mul(feat16[:tsz, tt, :], feat_ln[:tsz, tt, :], 1.0)
        for di in range(ndirs):
            for tt, (toff, tsz) in enumerate(TS):
                ps = ps1.tile([128, D], F32, tag="sps", name="ps")
                for kt, (koff, ksz) in enumerate(TS):
                    nc.tensor.matmul(ps[:tsz, :], P_sb[di][:ksz, kt, toff:toff + tsz],
                                     feat16[:ksz, kt, :], start=(kt == 0), stop=(kt == 1))
                nc.scalar.copy(res_t[di][:tsz, tt, :], ps[:tsz, :])

        # ================= depth loop, phase-clustered across directions ====
        mstate = [dict() for _ in range(ndirs)]

        for dep in range(ndepth):
            # --- A1: ln weights + residual update + LN xhat -> xlnT ---------
            for di in range(ndirs):
                st = mstate[di]
                st["lnwb"] = wpool.tile([128, 2, 2], F32, tag="lnwb", bufs=5,
                                        name="lnwb")
                for kd, (doff, dsz) in enumerate(KD):
                    nc.sync.dma_start(st["lnwb"][:dsz, kd, :],
                                      t_["lnwb"][di, dep, doff:doff + dsz, :])
                st["xlnT"] = apool.tile([128, 2, L], BF16, tag="xlnT", bufs=5,
                                        name="xlnT")

            if dep == 0:
                for di in range(ndirs):
                    st = mstate[di]
                    for kd, (doff, dsz) in enumerate(KD):
                        ps = ps1.tile([128, L], F32, tag="sps", name="ps")
                        for kt, (koff, ksz) in enumerate(TS):
                            nc.tensor.matmul(ps[:dsz, :],
                                             xhat0_16[:ksz, kt, doff:doff + dsz],
                                             P_sb[di][:ksz, kt, :],
                                             start=(kt == 0), stop=(kt == 1))
                        nc.scalar.activation(
                            st["xlnT"][:dsz, kd, :], ps[:dsz, :], AF.Identity,
                            scale=st["lnwb"][:dsz, kd, 0:1],
                            bias=st["lnwb"][:dsz, kd, 1:2])
            else:
                xh16s = []
                stats = []
                for di in range(ndirs):
                    res = res_t[di]
                    for tt, (toff, tsz) in enumerate(TS):
                        st6 = small.tile([128, 6], F32, tag="bn6", name="st6")
                        mv = small.tile([128, 2], F32, tag=f"bn2_{di}_{tt}", name="mv")
                        nc.vector.bn_stats(st6[:tsz], res[:tsz, tt, :])
                        nc.vector.bn_aggr(mv[:tsz], st6[:tsz])
                        stats.append(mv)
                # gather 8 vars -> one Ln + one Exp (one table load each)
                vars8 = small.tile([128, 8], F32, tag="vars8", name="vars8")
                for i, mv in enumerate(stats):
                    tsz = TS[i % 2][1]
                    nc.vector.tensor_scalar_add(vars8[:tsz, i:i + 1], mv[:tsz, 1:2],
                                                EPS)
                lnv8 = small.tile([128, 8], F32, tag="lnv8", name="lnv8")
                nc.scalar.activation(lnv8[:], vars8[:], AF.Ln)
                rstd8 = small.tile([128, 8], F32, tag="rstd8", name="rstd8")
                nc.scalar.activation(rstd8[:], lnv8[:], AF.Exp, scale=-0.5)
                # negmr[i] = -mean_i * rstd_i  (tiny DVE ops)
                negmr = small.tile([128, 8], F32, tag="negmr", name="negmr")
                for i, mv in enumerate(stats):
                    tsz = TS[i % 2][1]
                    nc.vector.tensor_scalar(
                        out=negmr[:tsz, i:i + 1], in0=mv[:tsz, 0:1],
                        scalar1=rstd8[:tsz, i:i + 1], scalar2=-1.0,
                        op0=ALU.mult, op1=ALU.mult)
                # xhat on ACT: Identity(res * rstd + negmr)
                for di in range(ndirs):
                    res = res_t[di]
                    xh16 = apool.tile([128, 2, D], BF16, tag="xh16", bufs=4)
                    for tt, (toff, tsz) in enumerate(TS):
                        i = di * 2 + tt
                        nc.scalar.activation(
                            xh16[:tsz, tt, :], res[:tsz, tt, :], AF.Identity,
                            scale=rstd8[:tsz, i:i + 1], bias=negmr[:tsz, i:i + 1])
                    xh16s.append(xh16)
                for di in range(ndirs):
                    st = mstate[di]
                    xh16 = xh16s[di]
                    for kd, (doff, dsz) in enumerate(KD):
                        ps = ps1.tile([128, L], BF16, tag="spsT", bufs=2, name="psT")
                        for tt, (toff, tsz) in enumerate(TS):
                            nc.tensor.transpose(ps[:dsz, toff:toff + tsz],
                                                xh16[:tsz, tt, doff:doff + dsz],
                                                ident[:tsz, :tsz])
                        nc.scalar.activation(
                            st["xlnT"][:dsz, kd, :], ps[:dsz, :], AF.Identity,
                            scale=st["lnwb"][:dsz, kd, 0:1],
                            bias=st["lnwb"][:dsz, kd, 1:2])

            # --- A2: in_proj + conv (DVE) + z evac (ACT copy) ---------------
            for di in range(ndirs):
                st = mstate[di]
                WinT_sb = wpool.tile([128, 2, 2 * D_IN], BF16, tag="WinT", bufs=2)
                for kd, (doff, dsz) in enumerate(KD):
                    nc.sync.dma_start(WinT_sb[:dsz, kd, :],
                                      t_["WinT"][di, dep, doff:doff + dsz, :])
                scal_sb = wpool.tile([128, NE, 6], F32, tag="scal", bufs=5)
                nc.sync.dma_start(
                    scal_sb[:], t_["scal"][di, dep].rearrange("(a p) s -> p a s", p=128))
                st["scal"] = scal_sb

                accz = apool.tile([128, 2, NE, L], BF16, tag="accz", bufs=4)
                acc = accz[:, 0]
                zsb = accz[:, 1]
                for ec in range(2 * NE):
                    ps = ps1.tile([128, L], F32, tag="sps", name="ps")
                    for kd, (doff, dsz) in enumerate(KD):
                        nc.tensor.matmul(ps[:, :],
                                         WinT_sb[:dsz, kd, ec * 128:(ec + 1) * 128],
                                         st["xlnT"][:dsz, kd, :],
                                         start=(kd == 0), stop=(kd == 1))
                    if ec < NE:
                        # causal depthwise conv reading u straight from PSUM
                        nc.vector.tensor_scalar(
                            out=acc[:, ec, :], in0=ps[:, :],
                            scalar1=scal_sb[:, ec, 3:4], scalar2=scal_sb[:, ec, 4:5],
                            op0=ALU.mult, op1=ALU.add)
                        for k in range(1, 4):
                            nc.vector.scalar_tensor_tensor(
                                out=acc[:, ec, k:L], in0=ps[:, 0:L - k],
                                scalar=scal_sb[:, ec, 3 - k:4 - k],
                                in1=acc[:, ec, k:L], op0=ALU.mult, op1=ALU.add)
                    else:
                        nc.vector.tensor_scalar_mul(zsb[:, ec - NE, :], ps[:, :], 1.0)
                st["accz"] = accz

            # --- A3/B1/B2 grouped: dir 0 end-to-end first, then dirs 1-3 ----
            for grp in ([0], list(range(1, ndirs))):
                for di in grp:
                    st = mstate[di]
                    u2sz = apool.tile([128, 2, NE, L], BF16, tag="u2sz", bufs=5)
                    nc.scalar.activation(u2sz[:].rearrange("p s a t -> p (s a t)"),
                                         st["accz"][:].rearrange("p s a t -> p (s a t)"),
                                         AF.Silu)
                    st["u2"] = u2sz[:, 0]
                    st["sz"] = u2sz[:, 1]

                for di in grp:
                    st = mstate[di]
                    WxT_sb = wpool.tile([128, NE, DT_R + 2 * N_ST], BF16, tag="WxT",
                                        bufs=5)
                    dtw2_sb = wpool.tile([DT_R + 1, NE, 128], BF16, tag="dtw2", bufs=5)
                    for ke in range(NE):
                        nc.sync.dma_start(WxT_sb[:, ke, :],
                                          t_["WxT"][di, dep, ke * 128:(ke + 1) * 128, :])
                        nc.sync.dma_start(dtw2_sb[:, ke, :],
                                          t_["dtw2"][di, dep, :, ke * 128:(ke + 1) * 128])
                    st["dtw2"] = dtw2_sb
                    st["WxT"] = WxT_sb

                    Bsb = small.tile([N_ST, L], BF16, tag="Bsb", bufs=4)
                    Csb = small.tile([N_ST, L], BF16, tag="Csb", bufs=4)
                    for si, (soff, ssz) in [(1, (DT_R, N_ST)), (2, (DT_R + N_ST, N_ST))]:
                        psx = ps1.tile([N_ST, L], F32, tag="spsx", bufs=2, name="psx")
                        for ke in range(NE):
                            nc.tensor.matmul(psx[:ssz, :],
                                             WxT_sb[:, ke, soff:soff + ssz],
                                             st["u2"][:, ke, :], start=(ke == 0),
                                             stop=(ke == NE - 1))
                        dst = Bsb if si == 1 else Csb
                        nc.scalar.copy(dst[:ssz, :], psx[:ssz, :])

                    bc_dram = dpool.tile([2, N_ST * L], BF16, tag="bc_dram", bufs=4)
                    nc.sync.dma_start(
                        bc_dram[0:1, :].rearrange("a (n t) -> (a n) t", t=L), Bsb[:, :])
                    nc.sync.dma_start(
                        bc_dram[1:2, :].rearrange("a (n t) -> (a n) t", t=L), Csb[:, :])
                    B_r = cpool.tile([128, N_ST, L], BF16, tag="Brep", bufs=3)
                    C_r = cpool.tile([128, N_ST, L], BF16, tag="Crep", bufs=2)
                    nc.sync.dma_start(B_r[:].rearrange("p n t -> p (n t)"),
                                      bc_dram[0:1, :].broadcast_to((128, N_ST * L)))
                    nc.sync.dma_start(C_r[:].rearrange("p n t -> p (n t)"),
                                      bc_dram[1:2, :].broadcast_to((128, N_ST * L)))
                    st["B_r"] = B_r
                    st["C_r"] = C_r

                    dtm = apool.tile([DT_R + 1, L], BF16, tag="dtm", bufs=5)
                    nc.sync.dma_start(dtm[DT_R:DT_R + 1, :], ones196[:])
                    psx = ps1.tile([N_ST, L], F32, tag="spsx", bufs=2, name="psx")
                    for ke in range(NE):
                        nc.tensor.matmul(psx[:DT_R, :], st["WxT"][:, ke, 0:DT_R],
                                         st["u2"][:, ke, :], start=(ke == 0),
                                         stop=(ke == NE - 1))
                    nc.scalar.copy(dtm[:DT_R, :], psx[:DT_R, :])
                    st["dtm"] = dtm

                for di in grp:
                    st = mstate[di]
                    Aneg_sb = wpool.tile([128, NE, N_ST], F32, tag="Aneg", bufs=5)
                    nc.sync.dma_start(
                        Aneg_sb[:],
                        t_["Aneg"][di, dep].rearrange("(a p) n -> p a n", p=128))
                    st["Aneg"] = Aneg_sb
                    spe = apool.tile([128, NE, L], BF16, tag="spe", bufs=4)
                    for ec in range(NE):
                        psd = ps1.tile([128, L], F32, tag="sps", name="ps")
                        nc.tensor.matmul(psd[:, :], st["dtw2"][:, ec, :],
                                         st["dtm"][:, :], start=True, stop=True)
                        nc.scalar.activation(spe[:, ec, :], psd[:, :], AF.Exp)
                    st["spe"] = spe
                for di in grp:
                    st = mstate[di]
                    delta = apool.tile([128, NE, L], BF16, tag="delta", bufs=5)
                    nc.scalar.activation(delta[:].rearrange("p a t -> p (a t)"),
                                         st["spe"][:].rearrange("p a t -> p (a t)"),
                                         AF.Ln, bias=1.0)
                    st["delta"] = delta
                for di in grp:
                    st = mstate[di]
                    v = apool.tile([128, NE, L], BF16, tag="v", bufs=5)
                    nc.vector.tensor_mul(v[:].rearrange("p a t -> p (a t)"),
                                         st["delta"][:].rearrange("p a t -> p (a t)"),
                                         st["u2"].rearrange("p a t -> p (a t)"))
                    st["v"] = v

            # --- B3: a-cube + scan + y (per ec, DVE+ACT pipelined) ----------
            for di in range(ndirs):
                st = mstate[di]
                y_sb = apool.tile([128, NE, L], BF16, tag="ysb", bufs=5)
                for ec in range(NE):
                    a_sb = cpool.tile([128, N_ST, L], F32, tag="acube", bufs=2)
                    nc.vector.memset(a_sb[:, :, 0:1], 0.0)
                    if di == 0 and ec == 0:
                        # boundary-fill path: da on DVE, one big exp on ACT
                        for n in range(N_ST):
                            nc.vector.tensor_scalar_mul(
                                a_sb[:, n, 1:L], st["delta"][:, ec, 1:L],
                                st["Aneg"][:, ec, n:n + 1])
                        nc.scalar.activation(a_sb[:, :, 1:L], a_sb[:, :, 1:L],
                                             AF.Exp)
                    else:
                        for n in range(N_ST):
                            nc.scalar.activation(a_sb[:, n, 1:L],
                                                 st["delta"][:, ec, 1:L],
                                                 AF.Exp, scale=st["Aneg"][:, ec, n:n + 1])
                    b_sb = cpool.tile([128, N_ST, L], BF16, tag="bg", bufs=2)
                    v_b = st["v"][:, ec, :].unsqueeze(1).broadcast_to((128, N_ST, L))
                    nc.vector.tensor_mul(b_sb[:], v_b, st["B_r"][:])
                    h_sb = cpool.tile([128, N_ST, L], BF16, tag="hcube", bufs=1)
                    nc.vector.tensor_tensor_scan(
                        out=h_sb[:].rearrange("p n t -> p (n t)"),
                        data0=a_sb[:].rearrange("p n t -> p (n t)"),
                        data1=b_sb[:].rearrange("p n t -> p (n t)"),
                        initial=0.0, op0=ALU.mult, op1=ALU.add)
                    g_sb = cpool.tile([128, N_ST, L], BF16, tag="bg", bufs=2)
                    nc.vector.tensor_mul(g_sb[:], h_sb[:], st["C_r"][:])
                    nc.vector.tensor_add(g_sb[:, 0:8, :], g_sb[:, 0:8, :],
                                         g_sb[:, 8:16, :])
                    nc.vector.tensor_add(g_sb[:, 0:4, :], g_sb[:, 0:4, :],
                                         g_sb[:, 4:8, :])
                    nc.vector.tensor_add(g_sb[:, 0:2, :], g_sb[:, 0:2, :],
                                         g_sb[:, 2:4, :])
                    nc.vector.tensor_add(y_sb[:, ec, :], g_sb[:, 0, :], g_sb[:, 1, :])
                st["y_sb"] = y_sb

            # --- B4: gate + out_proj -> hidden ------------------------------
            for di in range(ndirs):
                st = mstate[di]
                WoT_sb = wpool.tile([128, NE, D], BF16, tag="WoT", bufs=2)
                for ke in range(NE):
                    nc.sync.dma_start(WoT_sb[:, ke, :],
                                      t_["WoT"][di, dep, ke * 128:(ke + 1) * 128, :])
                y_sb = st["y_sb"]
                for ec in range(NE):
                    nc.vector.scalar_tensor_tensor(
                        out=y_sb[:, ec, :], in0=st["u2"][:, ec, :],
                        scalar=st["scal"][:, ec, 5:6], in1=y_sb[:, ec, :],
                        op0=ALU.mult, op1=ALU.add)
                nc.vector.tensor_mul(y_sb[:].rearrange("p a t -> p (a t)"),
                                     y_sb[:].rearrange("p a t -> p (a t)"),
                                     st["sz"].rearrange("p a t -> p (a t)"))
                for tt, (toff, tsz) in enumerate(TS):
                    pso = ps1.tile([128, D], F32, tag="sps", name="ps")
                    for ke in range(NE):
                        nc.tensor.matmul(pso[:tsz, :], y_sb[:, ke, toff:toff + tsz],
                                         WoT_sb[:, ke, :], start=(ke == 0),
                                         stop=(ke == NE - 1))
                    # residual update directly from PSUM (replaces hid tiles)
                    nc.vector.tensor_add(res_t[di][:tsz, tt, :],
                                         res_t[di][:tsz, tt, :], pso[:tsz, :])

        # ---- final = hidden + residual ; CrossMerge ----
        res16 = []
        for di in range(ndirs):
            r16 = apool.tile([128, 2, D], BF16, tag="res16", bufs=4, name="r16")
            for tt, (toff, tsz) in enumerate(TS):
                nc.vector.tensor_scalar_mul(r16[:tsz, tt, :], res_t[di][:tsz, tt, :],
                                            1.0)
            res16.append(r16)
        merged = state.tile([128, 2, D], F32, tag="merged")
        for tt, (toff, tsz) in enumerate(TS):
            ps = ps1.tile([128, D], F32, tag="sps", name="ps")
            nmm = ndirs * 2
            i = 0
            for di in range(ndirs):
                for kt, (koff, ksz) in enumerate(TS):
                    nc.tensor.matmul(ps[:tsz, :], PI_sb[di][:ksz, kt, toff:toff + tsz],
                                     res16[di][:ksz, kt, :], start=(i == 0),
                                     stop=(i == nmm - 1))
                    i += 1
            nc.scalar.copy(merged[:tsz, tt, :], ps[:tsz, :])

        # ---- out_norm LN + head LN ----
        xh = state.tile([128, 2, D], F32, tag="xh_final")
        emit_ln(xh, merged, "on")
        for tt, (toff, tsz) in enumerate(TS):
            nc.vector.tensor_mul(merged[:tsz, tt, :], xh[:tsz, tt, :], onw_r[:tsz, :])
            nc.vector.tensor_add(merged[:tsz, tt, :], merged[:tsz, tt, :], onb_r[:tsz, :])
        emit_ln(xh, merged, "hl")
        for tt, (toff, tsz) in enumerate(TS):
            nc.vector.tensor_mul(merged[:tsz, tt, :], xh[:tsz, tt, :], hlw_r[:tsz, :])
            nc.vector.tensor_add(merged[:tsz, tt, :], merged[:tsz, tt, :], hlb_r[:tsz, :])

        # ---- mean pool (x 1/L via ones value) ----
        psp = ps1.tile([1, D], F32, tag="spsx", bufs=2, name="psx")
        for kt, (koff, ksz) in enumerate(TS):
            nc.tensor.matmul(psp[:, :], onescol[:ksz, :], merged[:ksz, kt, :],
                             start=(kt == 0), stop=(kt == 1))
        pooled = small.tile([1, D], F32, tag="pooled")
        nc.scalar.copy(pooled[:], psp[:])
        pooledT = small.tile([128, 2, 1], BF16, tag="pooledT")
        for kd, (doff, dsz) in enumerate(KD):
            pst = ps1.tile([128, 1], F32, tag="spsx", bufs=2, name="psx")
            nc.tensor.transpose(pst[:dsz, :], pooled[:, doff:doff + dsz], identF[:1, :1])
            nc.scalar.copy(pooledT[:dsz, kd, :], pst[:dsz, :])

        # ---- head ----
        log_sb = small.tile([1, NCLS], F32, tag="logsb")
        for half in range(2):
            psh = ps1.tile([1, 500], F32, tag="spsx", bufs=2, name="psx")
            for kd, (doff, dsz) in enumerate(KD):
                nc.tensor.matmul(psh[:, :], pooledT[:dsz, kd, :],
                                 hwT_sb[:dsz, kd, half * 500:(half + 1) * 500],
                                 start=(kd == 0), stop=(kd == 1))
            nc.vector.tensor_add(log_sb[:, half * 500:(half + 1) * 500], psh[:, :],
                                 hb_sb[:, half * 500:(half + 1) * 500])
        nc.sync.dma_start(t_["logits"][:], log_sb[:])


# ============================== host side ==============================

_NC_CACHE = {}


def _get_nc():
    if "nc" not in _NC_CACHE:
        _NC_CACHE["nc"] = build_nc()
    return _NC_CACHE["nc"]


def _perm_matrices():
    idx = np.arange(L).reshape(H, W)
    perm0 = idx.reshape(-1)
    perm1 = idx.T.reshape(-1)
    perms = [perm0, perm1, perm0[::-1].copy(), perm1[::-1].copy()]
    P = np.zeros((4, L, L), np.float32)
    PI = np.zeros((4, L, L), np.float32)
    for di, pm in enumerate(perms):
        P[di, pm, np.arange(L)] = 1.0       # seq[t'] = sum_t P[t,t'] feat[t]
        PI[di] = P[di].T                     # merged[t] = sum_t' PI[t',t] out[t']
    return P, PI


def prep_inputs(inputs):
    """Host-side layout prep. Returns (shared weight map, per-core xcol list)."""
    bf = ml_dtypes.bfloat16
    g = {k: np.ascontiguousarray(np.asarray(v, dtype=np.float32))
         for k, v in inputs.items()}
    P, PI = _perm_matrices()
    lnwb = np.stack([g["ln_w"], g["ln_b"]], axis=-1)          # (4,8,192,2)
    scal = np.concatenate([g["conv_w"], g["conv_b"][..., None],
                           g["Dp"][..., None]], axis=-1)      # (4,8,384,6)
    dtw2 = np.concatenate([g["dt_w"].transpose(0, 1, 3, 2),
                           g["dt_b"][:, :, None, :]], axis=2)  # (4,8,13,384)
    shared = dict(
        pwT=np.ascontiguousarray(g["patch_w"].reshape(D, 768).T).astype(bf),
        pb=g["patch_b"], pe_g=g["pe_ln_w"], pe_b=g["pe_ln_b"],
        lnwb=np.ascontiguousarray(lnwb),
        WinT=np.ascontiguousarray(g["in_proj_w"].transpose(0, 1, 3, 2)).astype(bf),
        scal=np.ascontiguousarray(scal),
        WxT=np.ascontiguousarray(g["x_proj_w"].transpose(0, 1, 3, 2)).astype(bf),
        dtw2=np.ascontiguousarray(dtw2).astype(bf),
        Aneg=np.ascontiguousarray(-np.exp(g["A_log"])),
        WoT=np.ascontiguousarray(g["out_proj_w"].transpose(0, 1, 3, 2)).astype(bf),
        onw=g["out_norm_w"], onb=g["out_norm_b"],
        hlw=g["head_ln_w"], hlb=g["head_ln_b"],
        hwT=np.ascontiguousarray(g["head_w"].T).astype(bf), hb=g["head_b"],
        perm=P.astype(bf), permI=PI.astype(bf),
    )
    x = g["x"]
    xcols = []
    for b in range(x.shape[0]):
        xb = x[b]                                          # (3, 224, 224)
        c = xb.reshape(3, H, PATCH, W, PATCH)              # (3, i, pi, j, pj)
        col = c.transpose(0, 2, 4, 1, 3).reshape(768, L)   # (c,pi,pj),(i,j)
        xcols.append(np.ascontiguousarray(col).astype(bf))
    return shared, xcols


def kernel(**inputs):
    from concourse.bass_utils import run_bass_kernel_spmd

    nc = _get_nc()
    shared, xcols = prep_inputs(inputs)
    nb = len(xcols)
    in_maps = [dict(shared, xcol=xcols[b]) for b in range(nb)]
    res = run_bass_kernel_spmd(nc, in_maps, core_ids=list(range(nb)))
    out = np.stack([res.results[b]["logits"][0] for b in range(nb)])
    return out.astype(np.float32)


# revision 24
# speedup vs baseline: 1.2037x; 1.1787x over previous
"""Trainium2 Bass kernel for nn_BAAMamba (VMamba-style 4-direction Mamba classifier).

Sharding: pure data-parallel over batch - 8 cores x 1 image, each core runs the
full model on its image. No collectives.

v2 design (per-core, single NeuronCore), from measured op costs:
  - All cube elementwise ops in bf16 (DVE 2x mode); a-cube kept f32 so the
    scan decay chain has no compounding quantization.
  - n-reduction as a 4-level bf16 tree of tensor_adds (2.1us/ec) instead of
    strided tensor_reduce (5.6us/ec).
  - All matmuls in bf16 (weights host-cast), PSUM accumulates f32.
  - ACT ops clustered per depth by activation table; LN-stat ln/exp batched
    into one [128,8] op pair per depth (table reloads cost 1.3us each).
  - dt_b folded into the dt matmul via an appended ones row.
  - silu computed on ACT directly (Silu table) instead of sigmoid+DVE mul.
  - PSUM evacuations on ACT (copy/Identity-affine) to relieve DVE.
  - B/C broadcast cubes via bf16 DRAM round trip, double buffered.
  - Pool engine unused: it shares SBUF ports with DVE (measured 2-3x mutual
    slowdown) and cannot access PSUM; accumulating SWDGE DMA reduces race.
"""

import sys

import numpy as np

sys.path.insert(0, "/opt/trn_rl_repo")

import ml_dtypes  # noqa: E402

import concourse.bass as bass  # noqa: E402
import concourse.bacc as bacc  # noqa: E402
import concourse.tile as tile  # noqa: E402
from concourse import mybir  # noqa: E402

F32 = mybir.dt.float32
BF16 = mybir.dt.bfloat16
AF = mybir.ActivationFunctionType
ALU = mybir.AluOpType

B = 8
IMG = 224
PATCH = 16
D = 192
DEPTH = 8
H = IMG // PATCH
W = H
L = H * W                      # 196
D_IN = 384
N_ST = 16                      # D_STATE
DT_R = 12
NCLS = 1000
EPS = 1e-5

TS = [(0, 128), (128, L - 128)]          # t tiles (offset, size)
KD = [(0, 128), (128, D - 128)]          # d=192 contraction tiles
NE = D_IN // 128                         # 3 e-tiles
NDIR = 4


def build_nc(ndirs=NDIR, ndepth=DEPTH):
    nc = bacc.Bacc("TRN2")

    t_ = {}

    def din(name, shape, dt=BF16):
        t_[name] = nc.dram_tensor(name, shape, dt, kind="ExternalInput")

    din("xcol", (768, L))
    din("pwT", (768, D))
    din("pb", (D,), F32)
    din("pe_g", (D,), F32)
    din("pe_b", (D,), F32)
    din("lnwb", (4, DEPTH, D, 2), F32)           # [...,0]=w  [...,1]=b
    din("WinT", (4, DEPTH, D, 2 * D_IN))
    din("scal", (4, DEPTH, D_IN, 6), F32)        # convw(4) | convb | Dp
    din("WxT", (4, DEPTH, D_IN, DT_R + 2 * N_ST))
    din("dtw2", (4, DEPTH, DT_R + 1, D_IN))      # dt_w^T with dt_b row
    din("Aneg", (4, DEPTH, D_IN, N_ST), F32)
    din("WoT", (4, DEPTH, D_IN, D))
    din("onw", (D,), F32)
    din("onb", (D,), F32)
    din("hlw", (D,), F32)
    din("hlb", (D,), F32)
    din("hwT", (D, NCLS))
    din("hb", (NCLS,), F32)
    din("perm", (4, L, L))
    din("permI", (4, L, L))
    t_["logits"] = nc.dram_tensor("logits", (1, NCLS), F32, kind="ExternalOutput")

    with tile.TileContext(nc) as tc:
        _emit(nc, tc, t_, ndirs, ndepth)
    nc.compile()
    if not nc.is_finalized():
        nc.finalize()
    return nc


def _emit(nc, tc, t_, ndirs, ndepth):
    from contextlib import ExitStack

    with ExitStack() as ctx:
        consts = ctx.enter_context(tc.tile_pool(name="consts", bufs=1))
        wpool = ctx.enter_context(tc.tile_pool(name="wpool", bufs=2))
        state = ctx.enter_context(tc.tile_pool(name="state", bufs=1))
        apool = ctx.enter_context(tc.tile_pool(name="apool", bufs=2))
        small = ctx.enter_context(tc.tile_pool(name="small", bufs=3))
        cpool = ctx.enter_context(tc.tile_pool(name="cpool", bufs=2))
        ps1 = ctx.enter_context(tc.tile_pool(name="ps1", bufs=4, space="PSUM"))
        dpool = ctx.enter_context(tc.tile_pool(name="dpool", bufs=2, space="DRAM"))

        # ---- constants ----
        from concourse.masks import make_identity

        ident = consts.tile([128, 128], BF16)
        make_identity(nc, ident[:])
        identF = consts.tile([128, 128], F32)
        make_identity(nc, identF[:])

        pwT_sb = consts.tile([128, 6, D], BF16)
        col_sb = consts.tile([128, 6, L], BF16)
        for kt in range(6):
            nc.sync.dma_start(pwT_sb[:, kt, :], t_["pwT"][kt * 128:(kt + 1) * 128, :])
            nc.sync.dma_start(col_sb[:, kt, :], t_["xcol"][kt * 128:(kt + 1) * 128, :])

        P_sb = []
        PI_sb = []
        for di in range(ndirs):
            p = consts.tile([128, 2, L], BF16, tag=f"P{di}", name=f"P{di}")
            pi = consts.tile([128, 2, L], BF16, tag=f"PI{di}", name=f"PI{di}")
            for kt, (koff, ksz) in enumerate(TS):
                nc.sync.dma_start(p[:ksz, kt, :], t_["perm"][di, koff:koff + ksz, :])
                nc.sync.dma_start(pi[:ksz, kt, :], t_["permI"][di, koff:koff + ksz, :])
            P_sb.append(p)
            PI_sb.append(pi)

        def rep_vec(name):
            v = consts.tile([128, D], F32, tag=f"rep_{name}", name=f"rep_{name}")
            nc.sync.dma_start(v[:], t_[name][:].unsqueeze(0).broadcast_to((128, D)))
            return v

        pb_r = rep_vec("pb")
        peg_r = rep_vec("pe_g")
        peb_r = rep_vec("pe_b")
        onw_r = rep_vec("onw")
        onb_r = rep_vec("onb")
        hlw_r = rep_vec("hlw")
        hlb_r = rep_vec("hlb")

        hb_sb = consts.tile([1, NCLS], F32)
        nc.sync.dma_start(hb_sb[:], t_["hb"][:].unsqueeze(0))
        hwT_sb = consts.tile([128, 2, NCLS], BF16)
        for kd, (doff, dsz) in enumerate(KD):
            nc.sync.dma_start(hwT_sb[:dsz, kd, :], t_["hwT"][doff:doff + dsz, :])

        ones196 = consts.tile([1, L], BF16)
        nc.vector.memset(ones196[:], 1.0)
        onescol = consts.tile([128, 1], F32)
        nc.vector.memset(onescol[:], 1.0 / L)
        eps_t = consts.tile([128, 1], F32)
        nc.vector.memset(eps_t[:], EPS)

        # ---- helpers ----
        def emit_ln_stats(src_tt, tsz, tag):
            st6 = small.tile([128, 6], F32, tag="bn6", name="st6")
            mv = small.tile([128, 2], F32, tag=f"bn2_{tag}", name="mv")
            nc.vector.bn_stats(st6[:tsz], src_tt)
            nc.vector.bn_aggr(mv[:tsz], st6[:tsz])
            lnv = small.tile([128, 1], F32, tag="lnv", name="lnv")
            rstd = small.tile([128, 1], F32, tag=f"rstd_{tag}", name="rstd")
            nc.scalar.activation(lnv[:tsz], mv[:tsz, 1:2], AF.Ln, bias=eps_t[:tsz, :])
            nc.scalar.activation(rstd[:tsz], lnv[:tsz], AF.Exp, scale=-0.5)
            return mv, rstd

        def emit_ln(dst, src, tag):
            for tt, (toff, tsz) in enumerate(TS):
                mv, rstd = emit_ln_stats(src[:tsz, tt, :], tsz, tag)
                nc.vector.tensor_scalar(
                    out=dst[:tsz, tt, :], in0=src[:tsz, tt, :],
                    scalar1=mv[:tsz, 0:1], scalar2=rstd[:tsz, 0:1],
                    op0=ALU.subtract, op1=ALU.mult)

        # ---- patch embed ----
        feat_ln = state.tile([128, 2, D], F32, tag="feat_ln")
        for tt, (toff, tsz) in enumerate(TS):
            ps = ps1.tile([128, D], F32, tag="sps", name="ps")
            for kt in range(6):
                nc.tensor.matmul(ps[:tsz, :], col_sb[:, kt, toff:toff + tsz],
                                 pwT_sb[:, kt, :], start=(kt == 0), stop=(kt == 5))
            nc.vector.tensor_add(feat_ln[:tsz, tt, :], ps[:tsz, :], pb_r[:tsz, :])
        xhat0 = state.tile([128, 2, D], F32, tag="xhat0")
        emit_ln(xhat0, feat_ln, "pe")
        for tt, (toff, tsz) in enumerate(TS):
            nc.vector.tensor_mul(feat_ln[:tsz, tt, :], xhat0[:tsz, tt, :], peg_r[:tsz, :])
            nc.vector.tensor_add(feat_ln[:tsz, tt, :], feat_ln[:tsz, tt, :], peb_r[:tsz, :])
        # shared depth-0 block-LN xhat of feat_ln, in bf16 for the perm matmuls
        emit_ln(xhat0, feat_ln, "blk0")
        xhat0_16 = state.tile([128, 2, D], BF16, tag="xhat0_16")
        for tt, (toff, tsz) in enumerate(TS):
            nc.vector.tensor_scalar_mul(xhat0_16[:tsz, tt, :], xhat0[:tsz, tt, :], 1.0)

        # ---- per-direction persistent state ----
        res_t = [state.tile([128, 2, D], F32, tag=f"res{di}", name=f"res{di}")
                 for di in range(ndirs)]
        # residual init: res[di] = perm_di(feat_ln)
        feat16 = state.tile([128, 2, D], BF16, tag="feat16")
        for tt, (toff, tsz) in enumerate(TS):
            nc.vector.tensor_scalar_mul(feat16[:tsz, tt, :], feat_ln[:tsz, tt, :], 1.0)
        for di in range(ndirs):
            for tt, (toff, tsz) in enumerate(TS):
                ps = ps1.tile([128, D], F32, tag="sps", name="ps")
                for kt, (koff, ksz) in enumerate(TS):
                    nc.tensor.matmul(ps[:tsz, :], P_sb[di][:ksz, kt, toff:toff + tsz],
                                     feat16[:ksz, kt, :], start=(kt == 0), stop=(kt == 1))
                nc.scalar.copy(res_t[di][:tsz, tt, :], ps[:tsz, :])

        # ================= depth loop, phase-clustered across directions ====
        mstate = [dict() for _ in range(ndirs)]

        for dep in range(ndepth):
            # --- A1: ln weights + residual update + LN xhat -> xlnT ---------
            for di in range(ndirs):
                st = mstate[di]
                st["lnwb"] = wpool.tile([128, 2, 2], F32, tag="lnwb", bufs=5,
                                        name="lnwb")
                for kd, (doff, dsz) in enumerate(KD):
                    nc.sync.dma_start(st["lnwb"][:dsz, kd, :],
                                      t_["lnwb"][di, dep, doff:doff + dsz, :])
                st["xlnT"] = apool.tile([128, 2, L], BF16, tag="xlnT", bufs=5,
                                        name="xlnT")

            if dep == 0:
                for di in range(ndirs):
                    st = mstate[di]
                    for kd, (doff, dsz) in enumerate(KD):
                        ps = ps1.tile([128, L], F32, tag="sps", name="ps")
                        for kt, (koff, ksz) in enumerate(TS):
                            nc.tensor.matmul(ps[:dsz, :],
                                             xhat0_16[:ksz, kt, doff:doff + dsz],
                                             P_sb[di][:ksz, kt, :],
                                             start=(kt == 0), stop=(kt == 1))
                        nc.scalar.activation(
                            st["xlnT"][:dsz, kd, :], ps[:dsz, :], AF.Identity,
                            scale=st["lnwb"][:dsz, kd, 0:1],
                            bias=st["lnwb"][:dsz, kd, 1:2])
            else:
                xh16s = []
                stats = []
                for di in range(ndirs):
                    res = res_t[di]
                    for tt, (toff, tsz) in enumerate(TS):
                        st6 = small.tile([128, 6], F32, tag="bn6", name="st6")
                        mv = small.tile([128, 2], F32, tag=f"bn2_{di}_{tt}", name="mv")
                        nc.vector.bn_stats(st6[:tsz], res[:tsz, tt, :])
                        nc.vector.bn_aggr(mv[:tsz], st6[:tsz])
                        stats.append(mv)
                # gather 8 vars -> one Ln + one Exp (one table load each)
                vars8 = small.tile([128, 8], F32, tag="vars8", name="vars8")
                for i, mv in enumerate(stats):
                    tsz = TS[i % 2][1]
                    nc.vector.tensor_scalar_add(vars8[:tsz, i:i + 1], mv[:tsz, 1:2],
                                                EPS)
                lnv8 = small.tile([128, 8], F32, tag="lnv8", name="lnv8")
                nc.scalar.activation(lnv8[:], vars8[:], AF.Ln)
                rstd8 = small.tile([128, 8], F32, tag="rstd8", name="rstd8")
                nc.scalar.activation(rstd8[:], lnv8[:], AF.Exp, scale=-0.5)
                # negmr[i] = -mean_i * rstd_i  (tiny DVE ops)
                negmr = small.tile([128, 8], F32, tag="negmr", name="negmr")
                for i, mv in enumerate(stats):
                    tsz = TS[i % 2][1]
                    nc.vector.tensor_scalar(
                        out=negmr[:tsz, i:i + 1], in0=mv[:tsz, 0:1],
                        scalar1=rstd8[:tsz, i:i + 1], scalar2=-1.0,
                        op0=ALU.mult, op1=ALU.mult)
                # xhat on ACT: Identity(res * rstd + negmr)
                for di in range(ndirs):
                    res = res_t[di]
                    xh16 = apool.tile([128, 2, D], BF16, tag="xh16", bufs=4)
                    for tt, (toff, tsz) in enumerate(TS):
                        i = di * 2 + tt
                        nc.scalar.activation(
                            xh16[:tsz, tt, :], res[:tsz, tt, :], AF.Identity,
                            scale=rstd8[:tsz, i:i + 1], bias=negmr[:tsz, i:i + 1])
                    xh16s.append(xh16)
                for di in range(ndirs):
                    st = mstate[di]
                    xh16 = xh16s[di]
                    for kd, (doff, dsz) in enumerate(KD):
                        ps = ps1.tile([128, L], BF16, tag="spsT", bufs=2, name="psT")
                        for tt, (toff, tsz) in enumerate(TS):
                            nc.tensor.transpose(ps[:dsz, toff:toff + tsz],
                                                xh16[:tsz, tt, doff:doff + dsz],
                                                ident[:tsz, :tsz])
                        nc.scalar.activation(
                            st["xlnT"][:dsz, kd, :], ps[:dsz, :], AF.Identity,
                            scale=st["lnwb"][:dsz, kd, 0:1],
                            bias=st["lnwb"][:dsz, kd, 1:2])

            # --- A3/B1/B2 grouped: dir 0 end-to-end first, then dirs 1-3 ----
            for grp in ([0], list(range(1, ndirs))):
                for di in grp:
                    st = mstate[di]
                    WinT_sb = wpool.tile([128, 2, 2 * D_IN], BF16, tag="WinT", bufs=2)
                    for kd, (doff, dsz) in enumerate(KD):
                        nc.sync.dma_start(WinT_sb[:dsz, kd, :],
                                          t_["WinT"][di, dep, doff:doff + dsz, :])
                    scal_sb = wpool.tile([128, NE, 6], F32, tag="scal", bufs=5)
                    nc.sync.dma_start(
                        scal_sb[:],
                        t_["scal"][di, dep].rearrange("(a p) s -> p a s", p=128))
                    st["scal"] = scal_sb

                    acc = apool.tile([128, NE, L], BF16, tag="acc", bufs=4)
                    u2 = apool.tile([128, NE, L], BF16, tag="u2", bufs=5)
                    sz = apool.tile([128, NE, L], BF16, tag="sz", bufs=5)
                    for ec in range(2 * NE):
                        ps = ps1.tile([128, L], F32, tag="sps", name="ps")
                        for kd, (doff, dsz) in enumerate(KD):
                            nc.tensor.matmul(
                                ps[:, :], WinT_sb[:dsz, kd, ec * 128:(ec + 1) * 128],
                                st["xlnT"][:dsz, kd, :],
                                start=(kd == 0), stop=(kd == 1))
                        if ec < NE:
                            # causal depthwise conv reading u straight from PSUM
                            nc.vector.tensor_scalar(
                                out=acc[:, ec, :], in0=ps[:, :],
                                scalar1=scal_sb[:, ec, 3:4],
                                scalar2=scal_sb[:, ec, 4:5],
                                op0=ALU.mult, op1=ALU.add)
                            for k in range(1, 4):
                                nc.vector.scalar_tensor_tensor(
                                    out=acc[:, ec, k:L], in0=ps[:, 0:L - k],
                                    scalar=scal_sb[:, ec, 3 - k:4 - k],
                                    in1=acc[:, ec, k:L], op0=ALU.mult, op1=ALU.add)
                        else:
                            # z gate: silu straight from PSUM (no evacuation)
                            nc.scalar.activation(sz[:, ec - NE, :], ps[:, :], AF.Silu)
                    nc.scalar.activation(u2[:].rearrange("p a t -> p (a t)"),
                                         acc[:].rearrange("p a t -> p (a t)"),
                                         AF.Silu)
                    st["u2"] = u2
                    st["sz"] = sz

                for di in grp:
                    st = mstate[di]
                    WxT_sb = wpool.tile([128, NE, DT_R + 2 * N_ST], BF16, tag="WxT",
                                        bufs=5)
                    dtw2_sb = wpool.tile([DT_R + 1, NE, 128], BF16, tag="dtw2", bufs=5)
                    for ke in range(NE):
                        nc.sync.dma_start(WxT_sb[:, ke, :],
                                          t_["WxT"][di, dep, ke * 128:(ke + 1) * 128, :])
                        nc.sync.dma_start(dtw2_sb[:, ke, :],
                                          t_["dtw2"][di, dep, :, ke * 128:(ke + 1) * 128])
                    st["dtw2"] = dtw2_sb
                    st["WxT"] = WxT_sb

                    Bsb = small.tile([N_ST, L], BF16, tag="Bsb", bufs=4)
                    Csb = small.tile([N_ST, L], BF16, tag="Csb", bufs=4)
                    for si, (soff, ssz) in [(1, (DT_R, N_ST)), (2, (DT_R + N_ST, N_ST))]:
                        psx = ps1.tile([N_ST, L], F32, tag="spsx", bufs=2, name="psx")
                        for ke in range(NE):
                            nc.tensor.matmul(psx[:ssz, :],
                                             WxT_sb[:, ke, soff:soff + ssz],
                                             st["u2"][:, ke, :], start=(ke == 0),
                                             stop=(ke == NE - 1))
                        dst = Bsb if si == 1 else Csb
                        nc.scalar.copy(dst[:ssz, :], psx[:ssz, :])

                    bc_dram = dpool.tile([2, N_ST * L], BF16, tag="bc_dram", bufs=4)
                    nc.sync.dma_start(
                        bc_dram[0:1, :].rearrange("a (n t) -> (a n) t", t=L), Bsb[:, :])
                    nc.sync.dma_start(
                        bc_dram[1:2, :].rearrange("a (n t) -> (a n) t", t=L), Csb[:, :])
                    B_r = cpool.tile([128, N_ST, L], BF16, tag="Brep", bufs=3)
                    C_r = cpool.tile([128, N_ST, L], BF16, tag="Crep", bufs=2)
                    nc.sync.dma_start(B_r[:].rearrange("p n t -> p (n t)"),
                                      bc_dram[0:1, :].broadcast_to((128, N_ST * L)))
                    nc.sync.dma_start(C_r[:].rearrange("p n t -> p (n t)"),
                                      bc_dram[1:2, :].broadcast_to((128, N_ST * L)))
                    st["B_r"] = B_r
                    st["C_r"] = C_r

                    dtm = apool.tile([DT_R + 1, L], BF16, tag="dtm", bufs=5)
                    nc.sync.dma_start(dtm[DT_R:DT_R + 1, :], ones196[:])
                    psx = ps1.tile([N_ST, L], F32, tag="spsx", bufs=2, name="psx")
                    for ke in range(NE):
                        nc.tensor.matmul(psx[:DT_R, :], st["WxT"][:, ke, 0:DT_R],
                                         st["u2"][:, ke, :], start=(ke == 0),
                                         stop=(ke == NE - 1))
                    nc.scalar.copy(dtm[:DT_R, :], psx[:DT_R, :])
                    st["dtm"] = dtm

                for di in grp:
                    st = mstate[di]
                    Aneg_sb = wpool.tile([128, NE, N_ST], F32, tag="Aneg", bufs=5)
                    nc.sync.dma_start(
                        Aneg_sb[:],
                        t_["Aneg"][di, dep].rearrange("(a p) n -> p a n", p=128))
                    st["Aneg"] = Aneg_sb
                    spe = apool.tile([128, NE, L], BF16, tag="spe", bufs=4)
                    for ec in range(NE):
                        psd = ps1.tile([128, L], F32, tag="sps", name="ps")
                        nc.tensor.matmul(psd[:, :], st["dtw2"][:, ec, :],
                                         st["dtm"][:, :], start=True, stop=True)
                        nc.scalar.activation(spe[:, ec, :], psd[:, :], AF.Exp)
                    st["spe"] = spe
                for di in grp:
                    st = mstate[di]
                    delta = apool.tile([128, NE, L], BF16, tag="delta", bufs=5)
                    nc.scalar.activation(delta[:].rearrange("p a t -> p (a t)"),
                                         st["spe"][:].rearrange("p a t -> p (a t)"),
                                         AF.Ln, bias=1.0)
                    st["delta"] = delta
                for di in grp:
                    st = mstate[di]
                    v = apool.tile([128, NE, L], BF16, tag="v", bufs=5)
                    nc.vector.tensor_mul(v[:].rearrange("p a t -> p (a t)"),
                                         st["delta"][:].rearrange("p a t -> p (a t)"),
                                         st["u2"][:].rearrange("p a t -> p (a t)"))
                    st["v"] = v

            # --- B3: a-cube + scan + y (per ec, DVE+ACT pipelined) ----------
            for di in range(ndirs):
                st = mstate[di]
                y_sb = apool.tile([128, NE, L], BF16, tag="ysb", bufs=5)
                for ec in range(NE):
                    a_sb = cpool.tile([128, N_ST, L], F32, tag="acube", bufs=2)
                    nc.vector.memset(a_sb[:, :, 0:1], 0.0)
                    if di == 0 and ec == 0:
                        # boundary-fill path: da on DVE, one big exp on ACT
                        for n in range(N_ST):
                            nc.vector.tensor_scalar_mul(
                                a_sb[:, n, 1:L], st["delta"][:, ec, 1:L],
                                st["Aneg"][:, ec, n:n + 1])
                        nc.scalar.activation(a_sb[:, :, 1:L], a_sb[:, :, 1:L],
                                             AF.Exp)
                    else:
                        for n in range(N_ST):
                            nc.scalar.activation(a_sb[:, n, 1:L],
                                                 st["delta"][:, ec, 1:L],
                                                 AF.Exp, scale=st["Aneg"][:, ec, n:n + 1])
                    b_sb = cpool.tile([128, N_ST, L], BF16, tag="bg", bufs=2)
                    v_b = st["v"][:, ec, :].unsqueeze(1).broadcast_to((128, N_ST, L))
                    nc.vector.tensor_mul(b_sb[:], v_b, st["B_r"][:])
                    h_sb = cpool.tile([128, N_ST, L], BF16, tag="hcube", bufs=1)
                    nc.vector.tensor_tensor_scan(
                        out=h_sb[:].rearrange("p n t -> p (n t)"),
                        data0=a_sb[:].rearrange("p n t -> p (n t)"),
                        data1=b_sb[:].rearrange("p n t -> p (n t)"),
                        initial=0.0, op0=ALU.mult, op1=ALU.add)
                    g_sb = cpool.tile([128, N_ST, L], BF16, tag="bg", bufs=2)
                    nc.vector.tensor_mul(g_sb[:], h_sb[:], st["C_r"][:])
                    nc.vector.tensor_add(g_sb[:, 0:8, :], g_sb[:, 0:8, :],
                                         g_sb[:, 8:16, :])
                    nc.vector.tensor_add(g_sb[:, 0:4, :], g_sb[:, 0:4, :],
                                         g_sb[:, 4:8, :])
                    nc.vector.tensor_add(g_sb[:, 0:2, :], g_sb[:, 0:2, :],
                                         g_sb[:, 2:4, :])
                    nc.vector.tensor_add(y_sb[:, ec, :], g_sb[:, 0, :], g_sb[:, 1, :])
                st["y_sb"] = y_sb

            # --- B4: gate + out_proj -> hidden ------------------------------
            for di in range(ndirs):
                st = mstate[di]
                WoT_sb = wpool.tile([128, NE, D], BF16, tag="WoT", bufs=2)
                for ke in range(NE):
                    nc.sync.dma_start(WoT_sb[:, ke, :],
                                      t_["WoT"][di, dep, ke * 128:(ke + 1) * 128, :])
                y_sb = st["y_sb"]
                updt = apool.tile([128, NE, L], BF16, tag="updt", bufs=2)
                for ec in range(NE):
                    nc.vector.tensor_scalar_mul(updt[:, ec, :], st["u2"][:, ec, :],
                                                st["scal"][:, ec, 5:6])
                nc.vector.tensor_add(y_sb[:].rearrange("p a t -> p (a t)"),
                                     y_sb[:].rearrange("p a t -> p (a t)"),
                                     updt[:].rearrange("p a t -> p (a t)"))
                nc.vector.tensor_mul(y_sb[:].rearrange("p a t -> p (a t)"),
                                     y_sb[:].rearrange("p a t -> p (a t)"),
                                     st["sz"][:].rearrange("p a t -> p (a t)"))
                for tt, (toff, tsz) in enumerate(TS):
                    pso = ps1.tile([128, D], F32, tag="sps", name="ps")
                    for ke in range(NE):
                        nc.tensor.matmul(pso[:tsz, :], y_sb[:, ke, toff:toff + tsz],
                                         WoT_sb[:, ke, :], start=(ke == 0),
                                         stop=(ke == NE - 1))
                    # residual update directly from PSUM (replaces hid tiles)
                    nc.vector.tensor_add(res_t[di][:tsz, tt, :],
                                         res_t[di][:tsz, tt, :], pso[:tsz, :])

        # ---- final = hidden + residual ; CrossMerge ----
        res16 = []
        for di in range(ndirs):
            r16 = apool.tile([128, 2, D], BF16, tag="res16", bufs=4, name="r16")
            for tt, (toff, tsz) in enumerate(TS):
                nc.vector.tensor_scalar_mul(r16[:tsz, tt, :], res_t[di][:tsz, tt, :],
                                            1.0)
            res16.append(r16)
        merged = state.tile([128, 2, D], F32, tag="merged")
        for tt, (toff, tsz) in enumerate(TS):
            ps = ps1.tile([128, D], F32, tag="sps", name="ps")
            nmm = ndirs * 2
            i = 0
            for di in range(ndirs):
                for kt, (koff, ksz) in enumerate(TS):
                    nc.tensor.matmul(ps[:tsz, :], PI_sb[di][:ksz, kt, toff:toff + tsz],
                                     res16[di][:ksz, kt, :], start=(i == 0),
                                     stop=(i == nmm - 1))
                    i += 1
            nc.scalar.copy(merged[:tsz, tt, :], ps[:tsz, :])

        # ---- out_norm LN + head LN ----
        xh = state.tile([128, 2, D], F32, tag="xh_final")
        emit_ln(xh, merged, "on")
        for tt, (toff, tsz) in enumerate(TS):
            nc.vector.tensor_mul(merged[:tsz, tt, :], xh[:tsz, tt, :], onw_r[:tsz, :])
            nc.vector.tensor_add(merged[:tsz, tt, :], merged[:tsz, tt, :], onb_r[:tsz, :])
        emit_ln(xh, merged, "hl")
        for tt, (toff, tsz) in enumerate(TS):
            nc.vector.tensor_mul(merged[:tsz, tt, :], xh[:tsz, tt, :], hlw_r[:tsz, :])
            nc.vector.tensor_add(merged[:tsz, tt, :], merged[:tsz, tt, :], hlb_r[:tsz, :])

        # ---- mean pool (x 1/L via ones value) ----
        psp = ps1.tile([1, D], F32, tag="spsx", bufs=2, name="psx")
        for kt, (koff, ksz) in enumerate(TS):
            nc.tensor.matmul(psp[:, :], onescol[:ksz, :], merged[:ksz, kt, :],
                             start=(kt == 0), stop=(kt == 1))
        pooled = small.tile([1, D], F32, tag="pooled")
        nc.scalar.copy(pooled[:], psp[:])
        pooledT = small.tile([128, 2, 1], BF16, tag="pooledT")
        for kd, (doff, dsz) in enumerate(KD):
            pst = ps1.tile([128, 1], F32, tag="spsx", bufs=2, name="psx")
            nc.tensor.transpose(pst[:dsz, :], pooled[:, doff:doff + dsz], identF[:1, :1])
            nc.scalar.copy(pooledT[:dsz, kd, :], pst[:dsz, :])

        # ---- head ----
        log_sb = small.tile([1, NCLS], F32, tag="logsb")
        for half in range(2):
            psh = ps1.tile([1, 500], F32, tag="spsx", bufs=2, name="psx")
            for kd, (doff, dsz) in enumerate(KD):
                nc.tensor.matmul(psh[:, :], pooledT[:dsz, kd, :],
                                 hwT_sb[:dsz, kd, half * 500:(half + 1) * 500],
                                 start=(kd == 0), stop=(kd == 1))
            nc.vector.tensor_add(log_sb[:, half * 500:(half + 1) * 500], psh[:, :],
                                 hb_sb[:, half * 500:(half + 1) * 500])
        nc.sync.dma_start(t_["logits"][:], log_sb[:])


# ============================== host side ==============================

_NC_CACHE = {}


def _get_nc():
    if "nc" not in _NC_CACHE:
        _NC_CACHE["nc"] = build_nc()
    return _NC_CACHE["nc"]


def _perm_matrices():
    idx = np.arange(L).reshape(H, W)
    perm0 = idx.reshape(-1)
    perm1 = idx.T.reshape(-1)
    perms = [perm0, perm1, perm0[::-1].copy(), perm1[::-1].copy()]
    P = np.zeros((4, L, L), np.float32)
    PI = np.zeros((4, L, L), np.float32)
    for di, pm in enumerate(perms):
        P[di, pm, np.arange(L)] = 1.0       # seq[t'] = sum_t P[t,t'] feat[t]
        PI[di] = P[di].T                     # merged[t] = sum_t' PI[t',t] out[t']
    return P, PI


def prep_inputs(inputs):
    """Host-side layout prep. Returns (shared weight map, per-core xcol list)."""
    bf = ml_dtypes.bfloat16
    g = {k: np.ascontiguousarray(np.asarray(v, dtype=np.float32))
         for k, v in inputs.items()}
    P, PI = _perm_matrices()
    lnwb = np.stack([g["ln_w"], g["ln_b"]], axis=-1)          # (4,8,192,2)
    scal = np.concatenate([g["conv_w"], g["conv_b"][..., None],
                           g["Dp"][..., None]], axis=-1)      # (4,8,384,6)
    dtw2 = np.concatenate([g["dt_w"].transpose(0, 1, 3, 2),
                           g["dt_b"][:, :, None, :]], axis=2)  # (4,8,13,384)
    shared = dict(
        pwT=np.ascontiguousarray(g["patch_w"].reshape(D, 768).T).astype(bf),
        pb=g["patch_b"], pe_g=g["pe_ln_w"], pe_b=g["pe_ln_b"],
        lnwb=np.ascontiguousarray(lnwb),
        WinT=np.ascontiguousarray(g["in_proj_w"].transpose(0, 1, 3, 2)).astype(bf),
        scal=np.ascontiguousarray(scal),
        WxT=np.ascontiguousarray(g["x_proj_w"].transpose(0, 1, 3, 2)).astype(bf),
        dtw2=np.ascontiguousarray(dtw2).astype(bf),
        Aneg=np.ascontiguousarray(-np.exp(g["A_log"])),
        WoT=np.ascontiguousarray(g["out_proj_w"].transpose(0, 1, 3, 2)).astype(bf),
        onw=g["out_norm_w"], onb=g["out_norm_b"],
        hlw=g["head_ln_w"], hlb=g["head_ln_b"],
        hwT=np.ascontiguousarray(g["head_w"].T).astype(bf), hb=g["head_b"],
        perm=P.astype(bf), permI=PI.astype(bf),
    )
    x = g["x"]
    xcols = []
    for b in range(x.shape[0]):
        xb = x[b]                                          # (3, 224, 224)
        c = xb.reshape(3, H, PATCH, W, PATCH)              # (3, i, pi, j, pj)
        col = c.transpose(0, 2, 4, 1, 3).reshape(768, L)   # (c,pi,pj),(i,j)
        xcols.append(np.ascontiguousarray(col).astype(bf))
    return shared, xcols


def kernel(**inputs):
    from concourse.bass_utils import run_bass_kernel_spmd

    nc = _get_nc()
    shared, xcols = prep_inputs(inputs)
    nb = len(xcols)
    in_maps = [dict(shared, xcol=xcols[b]) for b in range(nb)]
    res = run_bass_kernel_spmd(nc, in_maps, core_ids=list(range(nb)))
    out = np.stack([res.results[b]["logits"][0] for b in range(nb)])
    return out.astype(np.float32)


# revision 25
# speedup vs baseline: 1.2294x; 1.0214x over previous
"""Trainium2 Bass kernel for nn_BAAMamba (VMamba-style 4-direction Mamba classifier).

Sharding: pure data-parallel over batch - 8 cores x 1 image, each core runs the
full model on its image. No collectives.

v2 design (per-core, single NeuronCore), from measured op costs:
  - All cube elementwise ops in bf16 (DVE 2x mode); a-cube kept f32 so the
    scan decay chain has no compounding quantization.
  - n-reduction as a 4-level bf16 tree of tensor_adds (2.1us/ec) instead of
    strided tensor_reduce (5.6us/ec).
  - All matmuls in bf16 (weights host-cast), PSUM accumulates f32.
  - ACT ops clustered per depth by activation table; LN-stat ln/exp batched
    into one [128,8] op pair per depth (table reloads cost 1.3us each).
  - dt_b folded into the dt matmul via an appended ones row.
  - silu computed on ACT directly (Silu table) instead of sigmoid+DVE mul.
  - PSUM evacuations on ACT (copy/Identity-affine) to relieve DVE.
  - B/C broadcast cubes via bf16 DRAM round trip, double buffered.
  - Pool engine unused: it shares SBUF ports with DVE (measured 2-3x mutual
    slowdown) and cannot access PSUM; accumulating SWDGE DMA reduces race.
"""

import sys

import numpy as np

sys.path.insert(0, "/opt/trn_rl_repo")

import ml_dtypes  # noqa: E402

import concourse.bass as bass  # noqa: E402
import concourse.bacc as bacc  # noqa: E402
import concourse.tile as tile  # noqa: E402
from concourse import mybir  # noqa: E402

F32 = mybir.dt.float32
BF16 = mybir.dt.bfloat16
AF = mybir.ActivationFunctionType
ALU = mybir.AluOpType

B = 8
IMG = 224
PATCH = 16
D = 192
DEPTH = 8
H = IMG // PATCH
W = H
L = H * W                      # 196
D_IN = 384
N_ST = 16                      # D_STATE
DT_R = 12
NCLS = 1000
EPS = 1e-5

TS = [(0, 128), (128, L - 128)]          # t tiles (offset, size)
KD = [(0, 128), (128, D - 128)]          # d=192 contraction tiles
NE = D_IN // 128                         # 3 e-tiles
NDIR = 4


def build_nc(ndirs=NDIR, ndepth=DEPTH):
    nc = bacc.Bacc("TRN2")

    t_ = {}

    def din(name, shape, dt=BF16):
        t_[name] = nc.dram_tensor(name, shape, dt, kind="ExternalInput")

    din("xcol", (768, L))
    din("pwT", (768, D))
    din("pb", (D,), F32)
    din("pe_g", (D,), F32)
    din("pe_b", (D,), F32)
    din("lnwb", (4, DEPTH, D, 2), F32)           # [...,0]=w  [...,1]=b
    din("WinT", (4, DEPTH, D, 2 * D_IN))
    din("scal", (4, DEPTH, D_IN, 6), F32)        # convw(4) | convb | Dp
    din("WxT", (4, DEPTH, D_IN, DT_R + 2 * N_ST))
    din("dtw2", (4, DEPTH, DT_R + 1, D_IN))      # dt_w^T with dt_b row
    din("Aneg", (4, DEPTH, D_IN, N_ST), F32)
    din("WoT", (4, DEPTH, D_IN, D))
    din("onw", (D,), F32)
    din("onb", (D,), F32)
    din("hlw", (D,), F32)
    din("hlb", (D,), F32)
    din("hwT", (D, NCLS))
    din("hb", (NCLS,), F32)
    din("perm", (4, L, L))
    din("permI", (4, L, L))
    t_["logits"] = nc.dram_tensor("logits", (1, NCLS), F32, kind="ExternalOutput")

    with tile.TileContext(nc) as tc:
        _emit(nc, tc, t_, ndirs, ndepth)
    nc.compile()
    if not nc.is_finalized():
        nc.finalize()
    return nc


def _emit(nc, tc, t_, ndirs, ndepth):
    from contextlib import ExitStack

    with ExitStack() as ctx:
        consts = ctx.enter_context(tc.tile_pool(name="consts", bufs=1))
        wpool = ctx.enter_context(tc.tile_pool(name="wpool", bufs=2))
        state = ctx.enter_context(tc.tile_pool(name="state", bufs=1))
        apool = ctx.enter_context(tc.tile_pool(name="apool", bufs=2))
        small = ctx.enter_context(tc.tile_pool(name="small", bufs=3))
        cpool = ctx.enter_context(tc.tile_pool(name="cpool", bufs=2))
        ps1 = ctx.enter_context(tc.tile_pool(name="ps1", bufs=4, space="PSUM"))
        dpool = ctx.enter_context(tc.tile_pool(name="dpool", bufs=2, space="DRAM"))

        # ---- constants ----
        from concourse.masks import make_identity

        ident = consts.tile([128, 128], BF16)
        make_identity(nc, ident[:])
        identF = consts.tile([128, 128], F32)
        make_identity(nc, identF[:])

        pwT_sb = consts.tile([128, 6, D], BF16)
        col_sb = consts.tile([128, 6, L], BF16)
        for kt in range(6):
            nc.sync.dma_start(pwT_sb[:, kt, :], t_["pwT"][kt * 128:(kt + 1) * 128, :])
            nc.sync.dma_start(col_sb[:, kt, :], t_["xcol"][kt * 128:(kt + 1) * 128, :])

        P_sb = []
        PI_sb = []
        for di in range(ndirs):
            p = consts.tile([128, 2, L], BF16, tag=f"P{di}", name=f"P{di}")
            pi = consts.tile([128, 2, L], BF16, tag=f"PI{di}", name=f"PI{di}")
            for kt, (koff, ksz) in enumerate(TS):
                nc.sync.dma_start(p[:ksz, kt, :], t_["perm"][di, koff:koff + ksz, :])
                nc.sync.dma_start(pi[:ksz, kt, :], t_["permI"][di, koff:koff + ksz, :])
            P_sb.append(p)
            PI_sb.append(pi)

        def rep_vec(name):
            v = consts.tile([128, D], F32, tag=f"rep_{name}", name=f"rep_{name}")
            nc.sync.dma_start(v[:], t_[name][:].unsqueeze(0).broadcast_to((128, D)))
            return v

        pb_r = rep_vec("pb")
        peg_r = rep_vec("pe_g")
        peb_r = rep_vec("pe_b")
        onw_r = rep_vec("onw")
        onb_r = rep_vec("onb")
        hlw_r = rep_vec("hlw")
        hlb_r = rep_vec("hlb")

        hb_sb = consts.tile([1, NCLS], F32)
        nc.sync.dma_start(hb_sb[:], t_["hb"][:].unsqueeze(0))
        hwT_sb = consts.tile([128, 2, NCLS], BF16)
        for kd, (doff, dsz) in enumerate(KD):
            nc.sync.dma_start(hwT_sb[:dsz, kd, :], t_["hwT"][doff:doff + dsz, :])

        ones196 = consts.tile([1, L], BF16)
        nc.vector.memset(ones196[:], 1.0)
        onescol = consts.tile([128, 1], F32)
        nc.vector.memset(onescol[:], 1.0 / L)
        eps_t = consts.tile([128, 1], F32)
        nc.vector.memset(eps_t[:], EPS)

        # ---- helpers ----
        def emit_ln_stats(src_tt, tsz, tag):
            st6 = small.tile([128, 6], F32, tag="bn6", name="st6")
            mv = small.tile([128, 2], F32, tag=f"bn2_{tag}", name="mv")
            nc.vector.bn_stats(st6[:tsz], src_tt)
            nc.vector.bn_aggr(mv[:tsz], st6[:tsz])
            lnv = small.tile([128, 1], F32, tag="lnv", name="lnv")
            rstd = small.tile([128, 1], F32, tag=f"rstd_{tag}", name="rstd")
            nc.scalar.activation(lnv[:tsz], mv[:tsz, 1:2], AF.Ln, bias=eps_t[:tsz, :])
            nc.scalar.activation(rstd[:tsz], lnv[:tsz], AF.Exp, scale=-0.5)
            return mv, rstd

        def emit_ln(dst, src, tag):
            for tt, (toff, tsz) in enumerate(TS):
                mv, rstd = emit_ln_stats(src[:tsz, tt, :], tsz, tag)
                nc.vector.tensor_scalar(
                    out=dst[:tsz, tt, :], in0=src[:tsz, tt, :],
                    scalar1=mv[:tsz, 0:1], scalar2=rstd[:tsz, 0:1],
                    op0=ALU.subtract, op1=ALU.mult)

        # ---- patch embed ----
        feat_ln = state.tile([128, 2, D], F32, tag="feat_ln")
        for tt, (toff, tsz) in enumerate(TS):
            ps = ps1.tile([128, D], F32, tag="sps", name="ps")
            for kt in range(6):
                nc.tensor.matmul(ps[:tsz, :], col_sb[:, kt, toff:toff + tsz],
                                 pwT_sb[:, kt, :], start=(kt == 0), stop=(kt == 5))
            nc.vector.tensor_add(feat_ln[:tsz, tt, :], ps[:tsz, :], pb_r[:tsz, :])
        xhat0 = state.tile([128, 2, D], F32, tag="xhat0")
        emit_ln(xhat0, feat_ln, "pe")
        for tt, (toff, tsz) in enumerate(TS):
            nc.vector.tensor_mul(feat_ln[:tsz, tt, :], xhat0[:tsz, tt, :], peg_r[:tsz, :])
            nc.vector.tensor_add(feat_ln[:tsz, tt, :], feat_ln[:tsz, tt, :], peb_r[:tsz, :])
        # shared depth-0 block-LN xhat of feat_ln, in bf16 for the perm matmuls
        emit_ln(xhat0, feat_ln, "blk0")
        xhat0_16 = state.tile([128, 2, D], BF16, tag="xhat0_16")
        for tt, (toff, tsz) in enumerate(TS):
            nc.vector.tensor_scalar_mul(xhat0_16[:tsz, tt, :], xhat0[:tsz, tt, :], 1.0)

        # ---- per-direction persistent state ----
        res_t = [state.tile([128, 2, D], F32, tag=f"res{di}", name=f"res{di}")
                 for di in range(ndirs)]
        # residual init: res[di] = perm_di(feat_ln)
        feat16 = state.tile([128, 2, D], BF16, tag="feat16")
        for tt, (toff, tsz) in enumerate(TS):
            nc.vector.tensor_scalar_mul(feat16[:tsz, tt, :], feat_ln[:tsz, tt, :], 1.0)
        for di in range(ndirs):
            for tt, (toff, tsz) in enumerate(TS):
                ps = ps1.tile([128, D], F32, tag="sps", name="ps")
                for kt, (koff, ksz) in enumerate(TS):
                    nc.tensor.matmul(ps[:tsz, :], P_sb[di][:ksz, kt, toff:toff + tsz],
                                     feat16[:ksz, kt, :], start=(kt == 0), stop=(kt == 1))
                nc.scalar.copy(res_t[di][:tsz, tt, :], ps[:tsz, :])

        # ================= depth loop, phase-clustered across directions ====
        mstate = [dict() for _ in range(ndirs)]

        for dep in range(ndepth):
            # --- A1: ln weights + residual update + LN xhat -> xlnT ---------
            for di in range(ndirs):
                st = mstate[di]
                st["lnwb"] = wpool.tile([128, 2, 2], F32, tag="lnwb", bufs=5,
                                        name="lnwb")
                for kd, (doff, dsz) in enumerate(KD):
                    nc.sync.dma_start(st["lnwb"][:dsz, kd, :],
                                      t_["lnwb"][di, dep, doff:doff + dsz, :])
                st["xlnT"] = apool.tile([128, 2, L], BF16, tag="xlnT", bufs=5,
                                        name="xlnT")

            if dep == 0:
                for di in range(ndirs):
                    st = mstate[di]
                    for kd, (doff, dsz) in enumerate(KD):
                        ps = ps1.tile([128, L], F32, tag="sps", name="ps")
                        for kt, (koff, ksz) in enumerate(TS):
                            nc.tensor.matmul(ps[:dsz, :],
                                             xhat0_16[:ksz, kt, doff:doff + dsz],
                                             P_sb[di][:ksz, kt, :],
                                             start=(kt == 0), stop=(kt == 1))
                        nc.scalar.activation(
                            st["xlnT"][:dsz, kd, :], ps[:dsz, :], AF.Identity,
                            scale=st["lnwb"][:dsz, kd, 0:1],
                            bias=st["lnwb"][:dsz, kd, 1:2])
            else:
                xh16s = []
                stats = []
                for di in range(ndirs):
                    res = res_t[di]
                    for tt, (toff, tsz) in enumerate(TS):
                        st6 = small.tile([128, 6], F32, tag="bn6", name="st6")
                        mv = small.tile([128, 2], F32, tag=f"bn2_{di}_{tt}", name="mv")
                        nc.vector.bn_stats(st6[:tsz], res[:tsz, tt, :])
                        nc.vector.bn_aggr(mv[:tsz], st6[:tsz])
                        stats.append(mv)
                # gather 8 vars -> one Ln + one Exp (one table load each)
                vars8 = small.tile([128, 8], F32, tag="vars8", name="vars8")
                for i, mv in enumerate(stats):
                    tsz = TS[i % 2][1]
                    nc.vector.tensor_scalar_add(vars8[:tsz, i:i + 1], mv[:tsz, 1:2],
                                                EPS)
                lnv8 = small.tile([128, 8], F32, tag="lnv8", name="lnv8")
                nc.scalar.activation(lnv8[:], vars8[:], AF.Ln)
                rstd8 = small.tile([128, 8], F32, tag="rstd8", name="rstd8")
                nc.scalar.activation(rstd8[:], lnv8[:], AF.Exp, scale=-0.5)
                # negmr[i] = -mean_i * rstd_i  (tiny DVE ops)
                negmr = small.tile([128, 8], F32, tag="negmr", name="negmr")
                for i, mv in enumerate(stats):
                    tsz = TS[i % 2][1]
                    nc.vector.tensor_scalar(
                        out=negmr[:tsz, i:i + 1], in0=mv[:tsz, 0:1],
                        scalar1=rstd8[:tsz, i:i + 1], scalar2=-1.0,
                        op0=ALU.mult, op1=ALU.mult)
                # xhat on ACT: Identity(res * rstd + negmr)
                for di in range(ndirs):
                    res = res_t[di]
                    xh16 = apool.tile([128, 2, D], BF16, tag="xh16", bufs=4)
                    for tt, (toff, tsz) in enumerate(TS):
                        i = di * 2 + tt
                        nc.scalar.activation(
                            xh16[:tsz, tt, :], res[:tsz, tt, :], AF.Identity,
                            scale=rstd8[:tsz, i:i + 1], bias=negmr[:tsz, i:i + 1])
                    xh16s.append(xh16)
                for di in range(ndirs):
                    st = mstate[di]
                    xh16 = xh16s[di]
                    for kd, (doff, dsz) in enumerate(KD):
                        ps = ps1.tile([128, L], BF16, tag="spsT", bufs=2, name="psT")
                        for tt, (toff, tsz) in enumerate(TS):
                            nc.tensor.transpose(ps[:dsz, toff:toff + tsz],
                                                xh16[:tsz, tt, doff:doff + dsz],
                                                ident[:tsz, :tsz])
                        nc.scalar.activation(
                            st["xlnT"][:dsz, kd, :], ps[:dsz, :], AF.Identity,
                            scale=st["lnwb"][:dsz, kd, 0:1],
                            bias=st["lnwb"][:dsz, kd, 1:2])

            # --- A2: in_proj + conv (DVE) + z evac (ACT copy) ---------------
            for di in range(ndirs):
                st = mstate[di]
                WinT_sb = wpool.tile([128, 2, 2 * D_IN], BF16, tag="WinT", bufs=2)
                for kd, (doff, dsz) in enumerate(KD):
                    nc.sync.dma_start(WinT_sb[:dsz, kd, :],
                                      t_["WinT"][di, dep, doff:doff + dsz, :])
                scal_sb = wpool.tile([128, NE, 6], F32, tag="scal", bufs=5)
                nc.sync.dma_start(
                    scal_sb[:], t_["scal"][di, dep].rearrange("(a p) s -> p a s", p=128))
                st["scal"] = scal_sb

                accz = apool.tile([128, 2, NE, L], BF16, tag="accz", bufs=4)
                acc = accz[:, 0]
                zsb = accz[:, 1]
                for ec in range(2 * NE):
                    ps = ps1.tile([128, L], F32, tag="sps", name="ps")
                    for kd, (doff, dsz) in enumerate(KD):
                        nc.tensor.matmul(ps[:, :],
                                         WinT_sb[:dsz, kd, ec * 128:(ec + 1) * 128],
                                         st["xlnT"][:dsz, kd, :],
                                         start=(kd == 0), stop=(kd == 1))
                    if ec < NE:
                        # causal depthwise conv reading u straight from PSUM
                        nc.vector.tensor_scalar(
                            out=acc[:, ec, :], in0=ps[:, :],
                            scalar1=scal_sb[:, ec, 3:4], scalar2=scal_sb[:, ec, 4:5],
                            op0=ALU.mult, op1=ALU.add)
                        for k in range(1, 4):
                            nc.vector.scalar_tensor_tensor(
                                out=acc[:, ec, k:L], in0=ps[:, 0:L - k],
                                scalar=scal_sb[:, ec, 3 - k:4 - k],
                                in1=acc[:, ec, k:L], op0=ALU.mult, op1=ALU.add)
                    else:
                        nc.vector.tensor_scalar_mul(zsb[:, ec - NE, :], ps[:, :], 1.0)
                st["accz"] = accz

            # --- A3/B1/B2 grouped: dir 0 end-to-end first, then dirs 1-3 ----
            for grp in ([0], list(range(1, ndirs))):
                for di in grp:
                    st = mstate[di]
                    u2sz = apool.tile([128, 2, NE, L], BF16, tag="u2sz", bufs=5)
                    nc.scalar.activation(u2sz[:].rearrange("p s a t -> p (s a t)"),
                                         st["accz"][:].rearrange("p s a t -> p (s a t)"),
                                         AF.Silu)
                    st["u2"] = u2sz[:, 0]
                    st["sz"] = u2sz[:, 1]

                for di in grp:
                    st = mstate[di]
                    WxT_sb = wpool.tile([128, NE, DT_R + 2 * N_ST], BF16, tag="WxT",
                                        bufs=5)
                    dtw2_sb = wpool.tile([DT_R + 1, NE, 128], BF16, tag="dtw2", bufs=5)
                    for ke in range(NE):
                        nc.sync.dma_start(WxT_sb[:, ke, :],
                                          t_["WxT"][di, dep, ke * 128:(ke + 1) * 128, :])
                        nc.sync.dma_start(dtw2_sb[:, ke, :],
                                          t_["dtw2"][di, dep, :, ke * 128:(ke + 1) * 128])
                    st["dtw2"] = dtw2_sb
                    st["WxT"] = WxT_sb

                    Bsb = small.tile([N_ST, L], BF16, tag="Bsb", bufs=4)
                    Csb = small.tile([N_ST, L], BF16, tag="Csb", bufs=4)
                    for si, (soff, ssz) in [(1, (DT_R, N_ST)), (2, (DT_R + N_ST, N_ST))]:
                        psx = ps1.tile([N_ST, L], F32, tag="spsx", bufs=2, name="psx")
                        for ke in range(NE):
                            nc.tensor.matmul(psx[:ssz, :],
                                             WxT_sb[:, ke, soff:soff + ssz],
                                             st["u2"][:, ke, :], start=(ke == 0),
                                             stop=(ke == NE - 1))
                        dst = Bsb if si == 1 else Csb
                        nc.scalar.copy(dst[:ssz, :], psx[:ssz, :])

                    bc_dram = dpool.tile([2, N_ST * L], BF16, tag="bc_dram", bufs=4)
                    nc.sync.dma_start(
                        bc_dram[0:1, :].rearrange("a (n t) -> (a n) t", t=L), Bsb[:, :])
                    nc.sync.dma_start(
                        bc_dram[1:2, :].rearrange("a (n t) -> (a n) t", t=L), Csb[:, :])
                    B_r = cpool.tile([128, N_ST, L], BF16, tag="Brep", bufs=3)
                    C_r = cpool.tile([128, N_ST, L], BF16, tag="Crep", bufs=2)
                    nc.sync.dma_start(B_r[:].rearrange("p n t -> p (n t)"),
                                      bc_dram[0:1, :].broadcast_to((128, N_ST * L)))
                    nc.sync.dma_start(C_r[:].rearrange("p n t -> p (n t)"),
                                      bc_dram[1:2, :].broadcast_to((128, N_ST * L)))
                    st["B_r"] = B_r
                    st["C_r"] = C_r

                    dtm = apool.tile([DT_R + 1, L], BF16, tag="dtm", bufs=5)
                    nc.sync.dma_start(dtm[DT_R:DT_R + 1, :], ones196[:])
                    psx = ps1.tile([N_ST, L], F32, tag="spsx", bufs=2, name="psx")
                    for ke in range(NE):
                        nc.tensor.matmul(psx[:DT_R, :], st["WxT"][:, ke, 0:DT_R],
                                         st["u2"][:, ke, :], start=(ke == 0),
                                         stop=(ke == NE - 1))
                    nc.scalar.copy(dtm[:DT_R, :], psx[:DT_R, :])
                    st["dtm"] = dtm

                for di in grp:
                    st = mstate[di]
                    Aneg_sb = wpool.tile([128, NE, N_ST], F32, tag="Aneg", bufs=5)
                    nc.sync.dma_start(
                        Aneg_sb[:],
                        t_["Aneg"][di, dep].rearrange("(a p) n -> p a n", p=128))
                    st["Aneg"] = Aneg_sb
                    spe = apool.tile([128, NE, L], BF16, tag="spe", bufs=4)
                    for ec in range(NE):
                        psd = ps1.tile([128, L], F32, tag="sps", name="ps")
                        nc.tensor.matmul(psd[:, :], st["dtw2"][:, ec, :],
                                         st["dtm"][:, :], start=True, stop=True)
                        nc.scalar.activation(spe[:, ec, :], psd[:, :], AF.Exp)
                    st["spe"] = spe
                for di in grp:
                    st = mstate[di]
                    delta = apool.tile([128, NE, L], BF16, tag="delta", bufs=5)
                    nc.scalar.activation(delta[:].rearrange("p a t -> p (a t)"),
                                         st["spe"][:].rearrange("p a t -> p (a t)"),
                                         AF.Ln, bias=1.0)
                    st["delta"] = delta
                for di in grp:
                    st = mstate[di]
                    v = apool.tile([128, NE, L], BF16, tag="v", bufs=5)
                    nc.vector.tensor_mul(v[:].rearrange("p a t -> p (a t)"),
                                         st["delta"][:].rearrange("p a t -> p (a t)"),
                                         st["u2"].rearrange("p a t -> p (a t)"))
                    st["v"] = v

            # --- B3: a-cube + scan + y (per ec, DVE+ACT pipelined) ----------
            for di in range(ndirs):
                st = mstate[di]
                y_sb = apool.tile([128, NE, L], BF16, tag="ysb", bufs=5)
                for ec in range(NE):
                    a_sb = cpool.tile([128, N_ST, L], F32, tag="acube", bufs=2)
                    nc.vector.memset(a_sb[:, :, 0:1], 0.0)
                    if di == 0 and ec == 0:
                        # boundary-fill path: da on DVE, one big exp on ACT
                        for n in range(N_ST):
                            nc.vector.tensor_scalar_mul(
                                a_sb[:, n, 1:L], st["delta"][:, ec, 1:L],
                                st["Aneg"][:, ec, n:n + 1])
                        nc.scalar.activation(a_sb[:, :, 1:L], a_sb[:, :, 1:L],
                                             AF.Exp)
                    else:
                        for n in range(N_ST):
                            nc.scalar.activation(a_sb[:, n, 1:L],
                                                 st["delta"][:, ec, 1:L],
                                                 AF.Exp, scale=st["Aneg"][:, ec, n:n + 1])
                    b_sb = cpool.tile([128, N_ST, L], BF16, tag="bg", bufs=2)
                    v_b = st["v"][:, ec, :].unsqueeze(1).broadcast_to((128, N_ST, L))
                    nc.vector.tensor_mul(b_sb[:], v_b, st["B_r"][:])
                    h_sb = cpool.tile([128, N_ST, L], BF16, tag="hcube", bufs=1)
                    nc.vector.tensor_tensor_scan(
                        out=h_sb[:].rearrange("p n t -> p (n t)"),
                        data0=a_sb[:].rearrange("p n t -> p (n t)"),
                        data1=b_sb[:].rearrange("p n t -> p (n t)"),
                        initial=0.0, op0=ALU.mult, op1=ALU.add)
                    g_sb = cpool.tile([128, N_ST, L], BF16, tag="bg", bufs=2)
                    nc.vector.tensor_mul(g_sb[:], h_sb[:], st["C_r"][:])
                    nc.vector.tensor_add(g_sb[:, 0:8, :], g_sb[:, 0:8, :],
                                         g_sb[:, 8:16, :])
                    nc.vector.tensor_add(g_sb[:, 0:4, :], g_sb[:, 0:4, :],
                                         g_sb[:, 4:8, :])
                    nc.vector.tensor_add(g_sb[:, 0:2, :], g_sb[:, 0:2, :],
                                         g_sb[:, 2:4, :])
                    nc.vector.tensor_add(y_sb[:, ec, :], g_sb[:, 0, :], g_sb[:, 1, :])
                st["y_sb"] = y_sb

            # --- B4: gate + out_proj -> hidden ------------------------------
            for di in range(ndirs):
                st = mstate[di]
                WoT_sb = wpool.tile([128, NE, D], BF16, tag="WoT", bufs=2)
                for ke in range(NE):
                    nc.sync.dma_start(WoT_sb[:, ke, :],
                                      t_["WoT"][di, dep, ke * 128:(ke + 1) * 128, :])
                y_sb = st["y_sb"]
                updt = apool.tile([128, NE, L], BF16, tag="updt", bufs=2)
                for ec in range(NE):
                    nc.vector.tensor_scalar_mul(updt[:, ec, :], st["u2"][:, ec, :],
                                                st["scal"][:, ec, 5:6])
                nc.vector.tensor_add(y_sb[:].rearrange("p a t -> p (a t)"),
                                     y_sb[:].rearrange("p a t -> p (a t)"),
                                     updt[:].rearrange("p a t -> p (a t)"))
                nc.vector.tensor_mul(y_sb[:].rearrange("p a t -> p (a t)"),
                                     y_sb[:].rearrange("p a t -> p (a t)"),
                                     st["sz"].rearrange("p a t -> p (a t)"))
                for tt, (toff, tsz) in enumerate(TS):
                    pso = ps1.tile([128, D], F32, tag="sps", name="ps")
                    for ke in range(NE):
                        nc.tensor.matmul(pso[:tsz, :], y_sb[:, ke, toff:toff + tsz],
                                         WoT_sb[:, ke, :], start=(ke == 0),
                                         stop=(ke == NE - 1))
                    # residual update directly from PSUM (replaces hid tiles)
                    nc.vector.tensor_add(res_t[di][:tsz, tt, :],
                                         res_t[di][:tsz, tt, :], pso[:tsz, :])

        # ---- final = hidden + residual ; CrossMerge ----
        res16 = []
        for di in range(ndirs):
            r16 = apool.tile([128, 2, D], BF16, tag="res16", bufs=4, name="r16")
            for tt, (toff, tsz) in enumerate(TS):
                nc.vector.tensor_scalar_mul(r16[:tsz, tt, :], res_t[di][:tsz, tt, :],
                                            1.0)
            res16.append(r16)
        merged = state.tile([128, 2, D], F32, tag="merged")
        for tt, (toff, tsz) in enumerate(TS):
            ps = ps1.tile([128, D], F32, tag="sps", name="ps")
            nmm = ndirs * 2
            i = 0
            for di in range(ndirs):
                for kt, (koff, ksz) in enumerate(TS):
                    nc.tensor.matmul(ps[:tsz, :], PI_sb[di][:ksz, kt, toff:toff + tsz],
                                     res16[di][:ksz, kt, :], start=(i == 0),
                                     stop=(i == nmm - 1))
                    i += 1
            nc.scalar.copy(merged[:tsz, tt, :], ps[:tsz, :])

        # ---- out_norm LN + head LN ----
        xh = state.tile([128, 2, D], F32, tag="xh_final")
        emit_ln(xh, merged, "on")
        for tt, (toff, tsz) in enumerate(TS):
            nc.vector.tensor_mul(merged[:tsz, tt, :], xh[:tsz, tt, :], onw_r[:tsz, :])
            nc.vector.tensor_add(merged[:tsz, tt, :], merged[:tsz, tt, :], onb_r[:tsz, :])
        emit_ln(xh, merged, "hl")
        for tt, (toff, tsz) in enumerate(TS):
            nc.vector.tensor_mul(merged[:tsz, tt, :], xh[:tsz, tt, :], hlw_r[:tsz, :])
            nc.vector.tensor_add(merged[:tsz, tt, :], merged[:tsz, tt, :], hlb_r[:tsz, :])

        # ---- mean pool (x 1/L via ones value) ----
        psp = ps1.tile([1, D], F32, tag="spsx", bufs=2, name="psx")
        for kt, (koff, ksz) in enumerate(TS):
            nc.tensor.matmul(psp[:, :], onescol[:ksz, :], merged[:ksz, kt, :],
                             start=(kt == 0), stop=(kt == 1))
        pooled = small.tile([1, D], F32, tag="pooled")
        nc.scalar.copy(pooled[:], psp[:])
        pooledT = small.tile([128, 2, 1], BF16, tag="pooledT")
        for kd, (doff, dsz) in enumerate(KD):
            pst = ps1.tile([128, 1], F32, tag="spsx", bufs=2, name="psx")
            nc.tensor.transpose(pst[:dsz, :], pooled[:, doff:doff + dsz], identF[:1, :1])
            nc.scalar.copy(pooledT[:dsz, kd, :], pst[:dsz, :])

        # ---- head ----
        log_sb = small.tile([1, NCLS], F32, tag="logsb")
        for half in range(2):
            psh = ps1.tile([1, 500], F32, tag="spsx", bufs=2, name="psx")
            for kd, (doff, dsz) in enumerate(KD):
                nc.tensor.matmul(psh[:, :], pooledT[:dsz, kd, :],
                                 hwT_sb[:dsz, kd, half * 500:(half + 1) * 500],
                                 start=(kd == 0), stop=(kd == 1))
            nc.vector.tensor_add(log_sb[:, half * 500:(half + 1) * 500], psh[:, :],
                                 hb_sb[:, half * 500:(half + 1) * 500])
        nc.sync.dma_start(t_["logits"][:], log_sb[:])


# ============================== host side ==============================

_NC_CACHE = {}


def _get_nc():
    if "nc" not in _NC_CACHE:
        _NC_CACHE["nc"] = build_nc()
    return _NC_CACHE["nc"]


def _perm_matrices():
    idx = np.arange(L).reshape(H, W)
    perm0 = idx.reshape(-1)
    perm1 = idx.T.reshape(-1)
    perms = [perm0, perm1, perm0[::-1].copy(), perm1[::-1].copy()]
    P = np.zeros((4, L, L), np.float32)
    PI = np.zeros((4, L, L), np.float32)
    for di, pm in enumerate(perms):
        P[di, pm, np.arange(L)] = 1.0       # seq[t'] = sum_t P[t,t'] feat[t]
        PI[di] = P[di].T                     # merged[t] = sum_t' PI[t',t] out[t']
    return P, PI


def prep_inputs(inputs):
    """Host-side layout prep. Returns (shared weight map, per-core xcol list)."""
    bf = ml_dtypes.bfloat16
    g = {k: np.ascontiguousarray(np.asarray(v, dtype=np.float32))
         for k, v in inputs.items()}
    P, PI = _perm_matrices()
    lnwb = np.stack([g["ln_w"], g["ln_b"]], axis=-1)          # (4,8,192,2)
    scal = np.concatenate([g["conv_w"], g["conv_b"][..., None],
                           g["Dp"][..., None]], axis=-1)      # (4,8,384,6)
    dtw2 = np.concatenate([g["dt_w"].transpose(0, 1, 3, 2),
                           g["dt_b"][:, :, None, :]], axis=2)  # (4,8,13,384)
    shared = dict(
        pwT=np.ascontiguousarray(g["patch_w"].reshape(D, 768).T).astype(bf),
        pb=g["patch_b"], pe_g=g["pe_ln_w"], pe_b=g["pe_ln_b"],
        lnwb=np.ascontiguousarray(lnwb),
        WinT=np.ascontiguousarray(g["in_proj_w"].transpose(0, 1, 3, 2)).astype(bf),
        scal=np.ascontiguousarray(scal),
        WxT=np.ascontiguousarray(g["x_proj_w"].transpose(0, 1, 3, 2)).astype(bf),
        dtw2=np.ascontiguousarray(dtw2).astype(bf),
        Aneg=np.ascontiguousarray(-np.exp(g["A_log"])),
        WoT=np.ascontiguousarray(g["out_proj_w"].transpose(0, 1, 3, 2)).astype(bf),
        onw=g["out_norm_w"], onb=g["out_norm_b"],
        hlw=g["head_ln_w"], hlb=g["head_ln_b"],
        hwT=np.ascontiguousarray(g["head_w"].T).astype(bf), hb=g["head_b"],
        perm=P.astype(bf), permI=PI.astype(bf),
    )
    x = g["x"]
    xcols = []
    for b in range(x.shape[0]):
        xb = x[b]                                          # (3, 224, 224)
        c = xb.reshape(3, H, PATCH, W, PATCH)              # (3, i, pi, j, pj)
        col = c.transpose(0, 2, 4, 1, 3).reshape(768, L)   # (c,pi,pj),(i,j)
        xcols.append(np.ascontiguousarray(col).astype(bf))
    return shared, xcols


def kernel(**inputs):
    from concourse.bass_utils import run_bass_kernel_spmd

    nc = _get_nc()
    shared, xcols = prep_inputs(inputs)
    nb = len(xcols)
    in_maps = [dict(shared, xcol=xcols[b]) for b in range(nb)]
    res = run_bass_kernel_spmd(nc, in_maps, core_ids=list(range(nb)))
    out = np.stack([res.results[b]["logits"][0] for b in range(nb)])
    return out.astype(np.float32)
